# revision 11
# baseline (speedup 1.0000x reference)
"""Trainium2 Bass kernel for a 2-layer LSTM binary classifier.

Model: xp0 = x @ Wih0.T + b0 ; layer0 LSTM ; xp1 = seq0 @ Wih1.T + b1 ;
layer1 LSTM ; out = h1_T @ Wfc.T + bfc.

Sharding: data-parallel over batch (64 -> 8 cores x 8 examples), all
weights replicated.  Per core:
  Phase 1 (interleaved with phase 2): big input GEMM, bias added via
    ones/onehot matmuls, output xp0 stored in SBUF as [128, gate, t, b].
    Chunk 0 (t<64) runs on fp8e4 weights (its xp0 error decays through
    >190 forget gates); later chunks use float32r (full PE rate at
    N>=256, ~tf32).  Startup is latency-tuned: HWDGE generation is ONE
    shared serialized resource (~630ns/DMA) and a DMA occupies its
    queue's SEQ until HWDGE accepts it, so all DMAs sit on the sync
    queue in priority order, constants ride in one packed blob, bulk
    tensors are flat [128,X] (the DMA cost model's descriptor unit is
    the lowest AP dim), ~4us of dummy matmuls pre-ramp the PE p-state,
    and pass 0 of the GEMM accumulates straight into the window-0 PSUM
    pair (layer-0 bias via the same K=4 onehot trick as layer 1) so the
    first recurrence step issues ~7.6us after kernel start.
  Phase 2: serial recurrence, the wall-clock driver (~1.79us/step chain
    latency x 258 macro-steps).  Gates live as [gate-dim on partitions,
    batch on free].  Per macro-step u: layer0 runs step u and layer1 runs
    step u-LAG so both layers share joint elementwise instructions.
    xp contributions are pre-accumulated into PSUM per W-step window
    (identity matmul for layer0's xp0; a single K=4 gate-onehot matmul
    broadcasts layer1's bias and must be the only start=True write to the
    bank -- start=True clears the whole bank's has_written bits).  Wih1 @
    h0(v) runs as per-step matmuls that depend on LAG-old data, so they
    execute off the critical path.  tanh-gate weights are pre-scaled 2x on
    the host so sigmoid covers the g-gate too (tanh(a) = 2*sig(2a)-1);
    the o-gate's sigmoid is a separate ACT op because it is only needed
    at the chain tail, so the on-chain sigmoid(i,f,g) is gated by just 6
    of 8 matmuls.  The 3-product cell update (tensor_mul + tensor_reduce
    originally) is ONE hand-built custom DVE op (LSTM_PAIRSUM3_ANT): a
    segmented product-scan over pages of 3 whose FSM (seed/steady/step,
    SUB_DIM_DONE reset -- the TENSOR_PAGED_MASK state-machine shape)
    resets the prefix sum at page boundaries; the out AP steers the two
    prefix elements onto junk/stale lanes and the page sum onto the
    opposite A/B state block's C lane.  Per-step chain: 6 bf16 matmuls ->
    sigmoid_ifg (ACT) -> PAIRSUM3 (DVE) -> tanh (ACT) -> h-mul (DVE,
    bf16 out) -> next step's matmuls; ~1.57us/step (was ~1.79us), every
    link at the cost model's per-op floor (ACT ops carry ~370ns of SBUF
    access latency, DVE ~120ns, PE matmuls a fixed 173ns drain).
"""

import numpy as np
import ml_dtypes

import concourse.bass as bass
import concourse.tile as tile
from concourse import bacc, mybir
from concourse.bass_utils import run_bass_kernel_spmd

# ---- custom DVE op: segmented product-sum (pages of 3) -------------------
# One Vector instruction replacing [tensor_mul(3 lanes) + tensor_reduce]:
# streams in0/in1 as [P, S, N=3] pages; per element computes p = in0*in1 and
# an inclusive per-page prefix sum (FSM: seed -> steady, SUB_DIM_DONE -> step
# resets the scan state to the current product, same state-machine shape as
# TENSOR_PAGED_MASK).  The page sum lands on the 3rd output element; the two
# prefix elements are steered to junk/stale lanes via the out AP strides.
from concourse.dve_spec import (_State, _Placement, _Stage, _assemble, PREV,
                                Spec, Src0, Src1, Zero, scan, COUNT_ONCE)
from concourse.dve_uop import (AluOp as DAlu, AluInp, Trigger, OutSel,
                               N_STAGES, DveOpSpec)
from concourse import dve_ops as _dve_ops
from concourse.dve_ops import DveOp


def _register_pairsum3():
    name = "LSTM_PAIRSUM3_ANT"
    if name in _dve_ops._SUB_OPCODE_FOR_NAME:
        return next(o for o in _dve_ops.OPS if o.name == name)
    empty = _Stage(DAlu.BYPASS, PREV)
    pipeline = [empty] * N_STAGES["v3"]
    pipeline[0] = _Stage(DAlu.MULTIPLY, Src0, Src1)
    pipeline[1] = _Stage(DAlu.ADD, AluInp.CURR_ALU_OUT, PREV)
    p = _Placement(pipeline=pipeline, node_stage={},
                   lane={Src0: 0, Src1: 1, Zero: 2},
                   out_sel=OutSel.ALU_OUT, accum_stage=None, captures=[])
    states = [
        _State(placement=p, trigger=COUNT_ONCE, repeat=1, next=(1, 0, 0),
               overrides={1: _Stage(DAlu.BYPASS, Zero, Zero)},
               write_out=False),
        _State(placement=p, consume=(True, True),
               trigger=(Trigger.SRC_TENSOR_DONE, Trigger.SUB_DIM_DONE,
                        Trigger.NONE), next=(0, 2, 0)),
        _State(placement=p, consume=(True, True), repeat=1,
               overrides={1: _Stage(DAlu.ADD, Zero, PREV)},
               trigger=(Trigger.SRC_TENSOR_DONE, Trigger.SUB_DIM_DONE,
                        Trigger.COUNT), next=(0, 2, 1)),
    ]
    uops = [_assemble(s) for s in states]
    for u in uops:
        u.validate("v3")
    opcode = max(_dve_ops._SUB_OPCODE_FOR_NAME.values()) + 1
    assert opcode < 0x20
    spec = Spec(body=scan(DAlu.ADD, Src0 * Src1))  # introspection-only
    op = DveOp(name, spec, subdim=True, uops_sha={})
    _dve_ops._COMPILE_CACHE[(name, "v3")] = DveOpSpec(
        name=name, opcode=opcode, uops=uops, rd1_en=True)
    _dve_ops.OPS.append(op)
    _dve_ops.CUSTOM_DVE_SPECS[name] = spec
    _dve_ops._SUB_OPCODE_FOR_NAME[name] = opcode
    return op


PAIRSUM3 = _register_pairsum3()

F32 = mybir.dt.float32
F32R = mybir.dt.float32r
BF16 = mybir.dt.bfloat16
F8 = mybir.dt.float8e4
AF = mybir.ActivationFunctionType

H = 128          # hidden
D = 2048         # input size
B = 64           # batch
T = 256          # seq len
NCORES = 8
BS = B // NCORES          # 8 examples per core
KT = D // 128             # 16 k-tiles of the input GEMM
NCHUNK = 4                # GEMM token chunks
CTOK = T * BS // NCHUNK   # 512 tokens per chunk
TW = CTOK // BS           # 64 timesteps per chunk
W = 8                     # recurrence window (psum burst granularity)
NW = T // W
LAG = 2                   # layer1 runs LAG steps behind layer0
GORD = [0, 1, 2, 3]       # our gate order [i,f,g,o] -> torch block index
P0TOK = [64, 64, 128, 256]  # chunk-0 GEMM pass sizes (tokens)
BLOBW = 1536              # packed-constant blob width (bf16 cols)


def _build_phase2_step(nc, u, P, hwin, hinit, whh0t_s, whh1t_s, wih1t_s,
                       ew, tcT):
    """Emit one macro-step: layer0 step u, layer1 step u-LAG."""
    w, s = divmod(u, W)
    active = []
    if u < T:
        active.append(0)
    if u >= LAG:
        active.append(1)

    def hprev(layer, step):
        if step == 0:
            return hinit[:, layer, :]
        pu = step - 1 + (LAG if layer == 1 else 0)
        return hwin[:, (pu // W) % 2, pu % W, layer, :]

    # layer1 input projection for step v=u-LAG: depends on h0(v), which was
    # produced LAG steps ago -> executes early on PE, off the critical path
    if 1 in active:
        v = u - LAG
        h0v = hwin[:, (v // W) % 2, v % W, 0, :]
        for gi in range(4):
            nc.tensor.matmul(P[:, 1, gi, s, :], wih1t_s[:, gi, :],
                             h0v, start=False, stop=False,
                             skip_group_check=True)

    # step matmuls
    for gi in (0, 1, 2, 3):
        for l in active:
            st = u if l == 0 else u - LAG
            lhs = whh0t_s if l == 0 else whh1t_s
            nc.tensor.matmul(P[:, l, gi, s, :], lhs[:, gi, :], hprev(l, st),
                             start=False, stop=True, skip_group_check=True)

    lo = active[0]
    ln = len(active)
    L = slice(lo, lo + ln)
    # g-gate weights/bias pre-scaled by 2 on host (tanh(a) = 2*sig(2a)-1)
    # and the cell state tracked as C = c/2 (exact), so the update is an
    # EQUAL-weight 3-product sum:  C' = sf*C + sg*si + si*(-0.5).
    # ew lanes (lane-major [128, lane, l, b]) double-buffer the state in A/B
    # blocks so ONE PAIRSUM3 custom op does products+sum in a single Vector
    # instruction: A = [K@0, C@1, si@2, sf@3, sg@4], B = [K@8, C@9, si@10,
    # sf@11, sg@12], so@6.  Even steps read A (in0 = lanes 2:5 overlapping
    # in1 = 0:3, the baseline trick) and the out AP (stride +2 lanes) steers
    # the two prefix elements to junk lanes 5,7 and the page sum to C-B@9;
    # odd steps read B and write (5,3,1) descending, landing C' on C-A@1.
    # Within-op write-then-read hazards: none (writes trail reads per
    # element, and junk targets are never read by the same op).
    A = (u % 2 == 0)
    gb = 2 if A else 10
    nc.scalar.activation(ew[:, gb:gb + 3, L, :],
                         P[:, L, 0:3, s, :].rearrange("p l g b -> p g l b"),
                         AF.Sigmoid)
    nc.scalar.activation(ew[:, 6, L, :], P[:, L, 3, s, :], AF.Sigmoid)
    nc.vector._custom_dve(
        PAIRSUM3,
        out=(ew[:, 5:10:2, L, :] if A
             else ew[:, 5:0:-2, L, :]).rearrange("p k l b -> p (l b) k"),
        in0=ew[:, gb:gb + 3, L, :].rearrange("p k l b -> p (l b) k"),
        in1=ew[:, gb - 2:gb + 1, L, :].rearrange("p k l b -> p (l b) k"))
    cl = 9 if A else 1
    nc.scalar.activation(tcT[:, L, :], ew[:, cl, L, :], AF.Tanh, scale=2.0)
    nc.vector.tensor_mul(hwin[:, w % 2, s, L, :], ew[:, 6, L, :],
                         tcT[:, L, :])


def build_program(debug_taps=False):
    nc = bacc.Bacc("TRN2", target_bir_lowering=False, debug=False,
                   enable_asserts=False)

    # ---- DRAM I/O ----
    xd = nc.dram_tensor("xp", [NCHUNK, 128, KT, CTOK], F32R,
                        kind="ExternalInput").ap()
    # all bulk tensors are flat [128, X] so their DMA APs have a large
    # innermost dim (the cost model's descriptor unit is the lowest AP dim)
    wih0d = nc.dram_tensor("wih0t", [128, KT * 512], F32R,
                           kind="ExternalInput").ap()
    # pass-0 x (first KT*64 cols) and the fp8 chunk-0 weights share one
    # dram tensor so the first startup DMA covers both (saves a HWDGE slot)
    wx0f8d = nc.dram_tensor("wx0f8", [128, KT * 64 + KT * 512], F8,
                            kind="ExternalInput").ap()
    xq_d = [None] + [nc.dram_tensor(f"x0q{i}", [128, KT * P0TOK[i]], F8,
                                    kind="ExternalInput").ap()
                     for i in range(1, len(P0TOK))]
    # weight blob (cols 0:512 whh0t, 512:1024 whh1t, 1024:1536 wih1t) is
    # DMA'd in two slices so whh0t lands early; every constant the first
    # recurrence window needs rides in the tiny cbias DMA (parts 0:4:
    # b0s4@0:128, onehot@128:384, b1s@1408:1536; part 0: b0s@384:896,
    # ones@896:1408) -- b1s must NOT wait on the blob, the u=0 layer-1
    # onehot matmul queues ahead of u=0's step matmuls on the PE.
    blobd = nc.dram_tensor("cblob", [128, BLOBW], BF16,
                           kind="ExternalInput").ap()
    cbiasd = nc.dram_tensor("cbias", [4, 1536], BF16,
                            kind="ExternalInput").ap()
    identd = nc.dram_tensor("ident", [128, 128], F32,
                            kind="ExternalInput").ap()
    wfcd = nc.dram_tensor("wfct", [128, 1], BF16, kind="ExternalInput").ap()
    bfcd = nc.dram_tensor("bfcb", [1, BS], F32, kind="ExternalInput").ap()
    yd = nc.dram_tensor("y", [BS, 1], F32, kind="ExternalOutput").ap()
    if debug_taps:
        dbg_xp0 = nc.dram_tensor("dbg_xp0", [128, 4, TW, BS], F32,
                                 kind="ExternalOutput").ap()
        dbg_hwin = nc.dram_tensor("dbg_hwin", [128, 2, W, 2, BS], BF16,
                                  kind="ExternalOutput").ap()
        dbg_c = nc.dram_tensor("dbg_c", [128, 2, BS], F32,
                               kind="ExternalOutput").ap()

    with tile.TileContext(nc) as tc, \
            tc.tile_pool(name="persist", bufs=1) as pp:
        # ---- persistent SBUF (bulk tensors flat, views for compute) ----
        wih0t_s = pp.tile([128, KT * 512], F32R, name="wih0t_s")
        wx0f8_s = pp.tile([128, KT * 64 + KT * 512], F8, name="wx0f8_s")
        wih0f8_s = wx0f8_s[:, KT * 64:]
        xq_s = [wx0f8_s[:, 0:KT * 64]] + [
            pp.tile([128, KT * n], F8, name=f"xq{i}_s")
            for i, n in enumerate(P0TOK) if i >= 1]
        blob_s = pp.tile([128, BLOBW], BF16, name="blob_s")
        cbias_s = pp.tile([4, 1536], BF16, name="cbias_s")
        ident_s = pp.tile([128, 128], F32, name="ident_s")
        wfct_s = pp.tile([128, 1], BF16, name="wfct_s")
        bfcb_s = pp.tile([1, BS], F32, name="bfcb_s")
        wones = pp.tile([1, 256], BF16, name="wones")
        y_sb = pp.tile([BS, 1], F32, name="y_sb")

        def wsl(wtile, k, g):  # [K=128, 128] lhsT slice of a flat w tile
            o = (k * 4 + g) * 128
            return wtile[:, o:o + 128]

        # constant views into the blob
        whh0t_s = blob_s[:, 0:512].rearrange("p (g u) -> p g u", g=4)
        whh1t_s = blob_s[:, 512:1024].rearrange("p (g u) -> p g u", g=4)
        wih1t_s = blob_s[:, 1024:1536].rearrange("p (g u) -> p g u", g=4)
        b1s_s = cbias_s[0:4, 1408:1536]
        b0s4_s = cbias_s[0:4, 0:128]
        onehot_s = cbias_s[0:4, 128:384].rearrange("p (g n) -> p g n", g=4)
        b0s_s = cbias_s[0:1, 384:896].rearrange("p (g u) -> p g u", g=4)
        ones_s = cbias_s[0:1, 896:1408]

        # Startup DMA schedule.  HWDGE generation is a single shared
        # serialized resource (~630ns/DMA) and a DMA instruction occupies
        # its queue's SEQ until HWDGE accepts it, so: recurrence-critical
        # DMAs first on sync in priority order, bulky weights on gpsimd
        # (SWDGE -- separate generator, keeps HWDGE free), and NOTHING on
        # the scalar/vector queues (their SEQs feed the recurrence).
        # Critical set for the first window: blob, wih0f8, xq0, ident.
        # single-queue DMA schedule in priority order (the DMA_ENGINES
        # transfer order is FIFO by generation-completion, so sync-queue
        # issue order IS the arrival order); chunk 0 uses the fp8 weights
        # throughout -- its xp0 error decays through >190 forget gates
        nc.sync.dma_start(cbias_s[:], cbiasd[:])
        nc.sync.dma_start(wx0f8_s[:, 0:3072], wx0f8d[:, 0:3072])
        for q in range(1, 4):
            o = KT * 64 + q * 2048
            nc.sync.dma_start(wx0f8_s[:, o:o + 2048], wx0f8d[:, o:o + 2048])
        nc.sync.dma_start(blob_s[:, 0:512], blobd[:, 0:512])
        nc.sync.dma_start(blob_s[:, 512:BLOBW], blobd[:, 512:BLOBW])
        nc.sync.dma_start(xq_s[1][:], xq_d[1][:])
        nc.sync.dma_start(ident_s[:], identd[:])
        nc.sync.dma_start(xq_s[2][:], xq_d[2][:])
        nc.sync.dma_start(xq_s[3][:], xq_d[3][:])
        nc.sync.dma_start(wfct_s[:], wfcd[:])
        nc.sync.dma_start(bfcb_s[:], bfcd[:])
        nc.sync.dma_start(wih0t_s[:], wih0d[:])

        # xp0 per chunk: [128, gate, t-local, b] fp32
        xp0_t = [pp.tile([128, 4, TW, BS], F32, name=f"xp0_{c}")
                 for c in range(NCHUNK)]

        # recurrence state: ew lane-major [128, lane, l, b]; lanes
        # [0:K-A, 1:C-A, 2:si-A, 3:sf-A, 4:sg-A, 5:junk, 6:so, 7:junk,
        #  8:K-B, 9:C-B, 10:si-B, 11:sf-B, 12:sg-B, 13:spare]
        hinit = pp.tile([128, 2, BS], BF16, name="hinit")
        hwin = pp.tile([128, 2, W, 2, BS], BF16, name="hwin")
        ew = pp.tile([128, 14, 2, BS], F32, name="ew")
        tcT = pp.tile([128, 2, BS], F32, name="tcT")

        nc.vector.memset(ew[:], 0.0)
        nc.vector.memset(ew[:, 0, :, :], -0.5)
        nc.vector.memset(ew[:, 8, :, :], -0.5)
        nc.vector.memset(hinit[:], 0.0)
        nc.vector.memset(wones[:], 0.0)
        # pre-warm the ACT function table (LoadActFuncSet ~1.3us) off-chain
        nc.scalar.activation(tcT[:, 0:1, :], ew[:, 1, 0:1, :], AF.Sigmoid)
        nc.scalar.activation(tcT[:, 0:1, :], ew[:, 1, 0:1, :], AF.Tanh)

        with (
            tc.tile_pool(name="xchunk", bufs=2) as x_pool,
            tc.tile_pool(name="gemm_ps", bufs=4, space="PSUM") as gemm_ps,
            tc.tile_pool(name="pair_ps", bufs=2, space="PSUM") as pair_ps,
        ):
            # ---- one chunk-0 GEMM pass: bias + k-MMs + copies ----
            def emit_pass_ops(i):
                ntok = P0TOK[i]
                toff = sum(P0TOK[:i])
                t0 = toff // BS
                tw_ = ntok // BS
                pg = []
                for g in range(4):
                    p = gemm_ps.tile([128, CTOK], F32, name="pg")
                    pg.append(p)
                    nc.tensor.matmul(p[:, 0:ntok], b0s_s[:, g, :],
                                     ones_s[:, 0:ntok],
                                     start=True, stop=False,
                                     skip_group_check=True)
                    yield 1
                for k in range(KT):
                    for g in range(4):
                        nc.tensor.matmul(
                            pg[g][:, 0:ntok], wsl(wih0f8_s, k, g),
                            xq_s[i][:, k * ntok:(k + 1) * ntok],
                            start=False, stop=(k == KT - 1),
                            skip_group_check=True)
                        yield 1
                for g in range(4):
                    dst = xp0_t[0][:, g, t0:t0 + tw_, :]
                    srcv = pg[g][:, 0:ntok].rearrange(
                        "p (t b) -> p t b", t=tw_)
                    nc.scalar.copy(dst, srcv)
                    yield 1

            # ---- GEMM op generator (pulled incrementally) ----
            def gemm_gen():
                # chunk-0 passes 1..3 (pass 0 runs in the prologue); x was
                # DMA'd up front, so only PE/copy ops are paced here
                for i in range(1, len(P0TOK)):
                    yield from emit_pass_ops(i)
                # chunks 1-3: f32r, per-k slab DMAs pace the matmuls
                for c in range(1, NCHUNK):
                    xt = x_pool.tile([128, KT, CTOK], F32R, name="xt")
                    for k in range(KT):
                        nc.sync.dma_start(xt[:, k, :], xd[c, :, k, :])
                        yield 1
                    pg = []
                    for g in range(4):
                        p = gemm_ps.tile([128, CTOK], F32, name="pg")
                        pg.append(p)
                        nc.tensor.matmul(p[:, :], b0s_s[:, g, :],
                                         ones_s[:, :],
                                         start=True, stop=False,
                                         skip_group_check=True)
                        yield 1
                    for k in range(KT):
                        for g in range(4):
                            nc.tensor.matmul(
                                pg[g][:, :], wsl(wih0t_s, k, g),
                                xt[:, k, :],
                                start=False, stop=(k == KT - 1),
                                skip_group_check=True)
                            yield 1
                    for g in range(4):
                        for hh in range(2):
                            tsl = slice(hh * TW // 2, (hh + 1) * TW // 2)
                            dst = xp0_t[c][:, g, tsl, :]
                            srcv = pg[g][:, hh * 256:(hh + 1) * 256].rearrange(
                                "p (t b) -> p t b", t=TW // 2)
                            nc.scalar.copy(dst, srcv)
                            yield 1

            gen = gemm_gen()

            def pull(n):
                for _ in range(n):
                    if next(gen, None) is None:
                        break

            # ---- prologue ----
            # PE warm-up: ~4us of dummy matmuls ramp the PE out of its cold
            # p-state so pass 0 runs at full clock the moment wih0f8 lands
            warm = gemm_ps.tile([128, CTOK], F32, name="pg")
            for _ in range(16):
                nc.tensor.matmul(warm[:, 0:256], wones[0:1, 0:128],
                                 wones[:, 0:256], start=True, stop=True,
                                 skip_group_check=True)
            # pass 0 (fp8, first window) accumulates STRAIGHT INTO the
            # window-0 PSUM pair: no xp0 store, no copies, no identity
            # injection on the critical path.  Layer 0's bias lands via the
            # same K=4 onehot trick as layer 1's (ONE start=True per bank).
            P0 = pair_ps.tile([128, 2, 4, 16, BS], F32, name="pairP")
            nc.tensor.matmul(P0[:, 0, :, 0:W, :], b0s4_s[:, :],
                             onehot_s[:, :, :],
                             start=True, stop=False, skip_group_check=True)
            for k in range(KT):
                for g in range(4):
                    nc.tensor.matmul(
                        P0[:, 0, g, 0:W, :].rearrange("p s b -> p (s b)"),
                        wsl(wih0f8_s, k, g),
                        xq_s[0][:, k * W * BS:(k + 1) * W * BS],
                        start=False, stop=False, skip_group_check=True)

            P = None
            for u in range(T + LAG):
                w, s = divmod(u, W)
                if s == 0:
                    P = P0 if u == 0 else pair_ps.tile(
                        [128, 2, 4, 16, BS], F32, name="pairP")
                    if u < T and u > 0:
                        c, lw = divmod(w, TW // W)
                        nc.tensor.matmul(
                            P[:, 0, :, 0:W, :],
                            ident_s[:, :],
                            xp0_t[c][:, :, lw * W:(lw + 1) * W, :],
                            start=True, stop=False, skip_group_check=True)
                    if u + W > LAG:
                        # whole-bank bias broadcast in ONE start=True matmul
                        # (start=True clears has_written for the full bank);
                        # Wih1 @ h0 is added per-step (off the critical path).
                        nc.tensor.matmul(
                            P[:, 1, :, 0:W, :], b1s_s[:, :], onehot_s[:, :, :],
                            start=True, stop=False, skip_group_check=True)
                _build_phase2_step(nc, u, P, hwin, hinit, whh0t_s, whh1t_s,
                                   wih1t_s, ew, tcT)
                # GEMM-op interleave AFTER the step's chain matmuls: the
                # in-order PE exec queue then holds [chain MMs (sem-gated),
                # GEMM MMs], so GEMM work fills the elementwise-phase idle
                # window and never straddles the h-sem release (straddling
                # 213ns f32r matmuls cost ~4us of stragglers otherwise).
                # Start at u>=4 so the queue never stalls on a matmul whose
                # x DMA is still in flight; 8/step drains chunk-0 passes
                # 1-3 early enough for their windows while chunks 1-3 pace
                # on their slab DMAs.
                if u >= 4:
                    pull(8 if u < 24 else 4)
                if debug_taps and u == 31:
                    nc.sync.dma_start(dbg_xp0[:], xp0_t[0][:])
                    nc.sync.dma_start(dbg_hwin[:], hwin[:])
                    nc.sync.dma_start(dbg_c[:], cC[:])

            pull(10000)  # drain any leftovers (shouldn't be needed)

            # ---- final fc: bias folded in as a K=1 matmul (ident[0,0]
            # supplies the f32 one), y DMA'd straight from PSUM ----
            fcp = gemm_ps.tile([BS, 1], F32, name="pg")
            nc.tensor.matmul(fcp[:, :], hwin[:, (T + LAG - 1) // W % 2,
                                             (T + LAG - 1) % W, 1, :],
                             wfct_s[:, :], start=True, stop=False,
                             skip_group_check=True)
            nc.tensor.matmul(fcp[:, :], bfcb_s[:, :], ident_s[0:1, 0:1],
                             start=False, stop=True, skip_group_check=True)
            nc.vector.tensor_copy(y_sb[:, :], fcp[:, :])
            nc.sync.dma_start(yd[:], y_sb[:])

    nc.compile()
    return nc


_PROG = None


def _get_program():
    global _PROG
    if _PROG is None:
        _PROG = build_program()
    return _PROG


def prep_inputs(x, Wih0, Whh0, bih0, bhh0, Wih1, Whh1, bih1, bhh1, Wfc, bfc):
    """Host-side layout prep -> per-core in_maps."""
    bf = ml_dtypes.bfloat16
    f8 = ml_dtypes.float8_e4m3
    x = np.asarray(x, np.float32)

    # weights: [4H, K] -> [K(part), gate(ours), unit]
    def gate_T(Wmat):  # [512, K] -> [K, 4, 128] in our gate order
        A = np.asarray(Wmat, np.float32).reshape(4, 128, -1)  # tg, j, k
        A = A.transpose(2, 0, 1)[:, GORD, :]                  # k, ours, j
        A = A.copy()
        A[:, 2, :] *= 2.0  # tanh-gate folded 2x (tanh(a)=2*sig(2a)-1)
        return np.ascontiguousarray(A)

    wih0t = gate_T(Wih0).reshape(KT, 128, 4, 128).transpose(1, 0, 2, 3)
    wih0t = np.ascontiguousarray(wih0t, np.float32)           # [128,KT,4,128]
    whh0t = gate_T(Whh0).astype(bf)                           # [128,4,128]
    whh1t = gate_T(Whh1).astype(bf)
    wih1t = gate_T(Wih1).astype(bf)

    b0 = (np.asarray(bih0) + np.asarray(bhh0)).astype(np.float32)
    b1 = (np.asarray(bih1) + np.asarray(bhh1)).astype(np.float32)
    b0s = b0.reshape(4, 128)[GORD].copy()
    b0s[2] *= 2.0                                             # [4,128]
    b1g4 = b1.reshape(4, 128)[GORD].copy()
    b1g4[2] *= 2.0                                            # [4,128]
    ident = np.eye(128, dtype=np.float32)
    wfct = np.asarray(Wfc, np.float32).T.astype(bf)           # [128,1]
    bfcb = np.full((1, BS), np.asarray(bfc, np.float32)[0], np.float32)

    # packed constant blob (single startup DMA); layout must match the
    # blob_s views in build_program
    blob = np.zeros((128, BLOBW), np.float32)
    blob[:, 0:512] = whh0t.reshape(128, 512)
    blob[:, 512:1024] = whh1t.reshape(128, 512)
    blob[:, 1024:1536] = wih1t.reshape(128, 512)
    blob = blob.astype(bf)
    cbias = np.zeros((4, 1536), np.float32)
    cbias[0:4, 0:128] = b0s
    cbias[0:4, 128:384] = np.repeat(np.eye(4, dtype=np.float32),
                                    W * BS).reshape(4, 4 * W * BS)
    cbias[0, 384:896] = b0s.reshape(512)
    cbias[0, 896:1408] = 1.0
    cbias[0:4, 1408:1536] = b1g4
    cbias = cbias.astype(bf)

    wih0flat = wih0t.reshape(128, KT * 512)
    common = dict(wih0t=wih0flat,
                  cblob=blob, cbias=cbias, ident=ident, wfct=wfct,
                  bfcb=bfcb)

    offs = np.cumsum([0] + P0TOK)
    in_maps = []
    for c in range(NCORES):
        xs = x[c * BS:(c + 1) * BS]                           # [BS, T, D]
        xt = xs.transpose(2, 1, 0).reshape(D, T * BS)         # [d, tok(t,b)]
        xpre = (xt.reshape(KT, 128, NCHUNK, CTOK)
                .transpose(2, 1, 0, 3))                       # [c,128,k,tok]
        m = {"xp": np.ascontiguousarray(xpre, np.float32), **common}
        for i in range(len(P0TOK)):
            seg = np.ascontiguousarray(xpre[0][:, :, offs[i]:offs[i + 1]])
            seg = seg.astype(f8).reshape(128, KT * P0TOK[i])
            if i == 0:
                m["wx0f8"] = np.concatenate(
                    [seg, wih0flat.astype(f8)], axis=1)
            else:
                m[f"x0q{i}"] = seg
        in_maps.append(m)
    return in_maps


def run(inputs, **kw):
    nc = _get_program()
    in_maps = prep_inputs(**inputs)
    res = run_bass_kernel_spmd(nc, in_maps, core_ids=list(range(NCORES)), **kw)
    y = np.concatenate([res.results[c]["y"] for c in range(NCORES)], axis=0)
    return y.astype(np.float32), res


def kernel(**inputs):
    y, _ = run(inputs)
    return y


if __name__ == "__main__":
    import sys
    if "--sim" in sys.argv:
        import trails.perfetto as _tp
        if not hasattr(_tp.LazyPerfetto, "add_counter"):
            def _add_counter(self, proc, track, ts_, val):
                self.update_counter(proc, track, int(ts_), float(val),
                                    unit="ns")
            _tp.LazyPerfetto.add_counter = _add_counter
        for _m in ("enable_explicit_ordering", "reserve_process_order"):
            if not hasattr(_tp.LazyPerfetto, _m):
                setattr(_tp.LazyPerfetto, _m,
                        lambda self, *a, **k: None)
        from concourse.timeline_sim import TimelineSim
        nc = _get_program()
        ts = TimelineSim(nc, trace="--trace" in sys.argv)
        dur = ts.simulate()
        print(f"TimelineSim predicted duration: {dur:.0f} ns")
        if ts.perfetto is not None:
            ts.perfetto.save("/root/problem/timeline.pftrace")
            print("wrote /root/problem/timeline.pftrace")



# revision 16
# speedup vs baseline: 1.0001x; 1.0001x over previous
"""Trainium2 Bass kernel for a 2-layer LSTM binary classifier.

Model: xp0 = x @ Wih0.T + b0 ; layer0 LSTM ; xp1 = seq0 @ Wih1.T + b1 ;
layer1 LSTM ; out = h1_T @ Wfc.T + bfc.

Sharding: data-parallel over batch (64 -> 8 cores x 8 examples), all
weights replicated.  Per core:
  Phase 1 (interleaved with phase 2): big input GEMM, bias added via
    ones/onehot matmuls, output xp0 stored in SBUF as [128, gate, t, b].
    Chunk 0 (t<64) runs on fp8e4 weights (its xp0 error decays through
    >190 forget gates); later chunks use float32r (full PE rate at
    N>=256, ~tf32).  Startup is latency-tuned: HWDGE generation is ONE
    shared serialized resource (~630ns/DMA) and a DMA occupies its
    queue's SEQ until HWDGE accepts it, so all DMAs sit on the sync
    queue in priority order, constants ride in one packed blob, bulk
    tensors are flat [128,X] (the DMA cost model's descriptor unit is
    the lowest AP dim), ~4us of dummy matmuls pre-ramp the PE p-state,
    and pass 0 of the GEMM accumulates straight into the window-0 PSUM
    pair (layer-0 bias via the same K=4 onehot trick as layer 1) so the
    first recurrence step issues ~7.6us after kernel start.
  Phase 2: serial recurrence, the wall-clock driver (~1.79us/step chain
    latency x 258 macro-steps).  Gates live as [gate-dim on partitions,
    batch on free].  Per macro-step u: layer0 runs step u and layer1 runs
    step u-LAG so both layers share joint elementwise instructions.
    xp contributions are pre-accumulated into PSUM per W-step window
    (identity matmul for layer0's xp0; a single K=4 gate-onehot matmul
    broadcasts layer1's bias and must be the only start=True write to the
    bank -- start=True clears the whole bank's has_written bits).  Wih1 @
    h0(v) runs as per-step matmuls that depend on LAG-old data, so they
    execute off the critical path.  tanh-gate weights are pre-scaled 2x on
    the host so sigmoid covers the g-gate too (tanh(a) = 2*sig(2a)-1);
    the o-gate's sigmoid is a separate ACT op because it is only needed
    at the chain tail, so the on-chain sigmoid(i,f,g) is gated by just 6
    of 8 matmuls.  The 3-product cell update (tensor_mul + tensor_reduce
    originally) is ONE hand-built custom DVE op (LSTM_PAIRSUM3_ANT): a
    segmented product-scan over pages of 3 whose FSM (seed/steady/step,
    SUB_DIM_DONE reset -- the TENSOR_PAGED_MASK state-machine shape)
    resets the prefix sum at page boundaries; the out AP steers the two
    prefix elements onto junk/stale lanes and the page sum onto the
    opposite A/B state block's C lane.  Per-step chain: 6 bf16 matmuls ->
    sigmoid_ifg (ACT) -> PAIRSUM3 (DVE) -> tanh (ACT) -> h-mul (DVE,
    bf16 out) -> next step's matmuls; ~1.57us/step (was ~1.79us), every
    link at the cost model's per-op floor (ACT ops carry ~370ns of SBUF
    access latency, DVE ~120ns, PE matmuls a fixed 173ns drain).
"""

import numpy as np
import ml_dtypes

import concourse.bass as bass
import concourse.tile as tile
from concourse import bacc, mybir
from concourse.bass_utils import run_bass_kernel_spmd

# ---- custom DVE op: segmented product-sum (pages of 3) -------------------
# One Vector instruction replacing [tensor_mul(3 lanes) + tensor_reduce]:
# streams in0/in1 as [P, S, N=3] pages; per element computes p = in0*in1 and
# an inclusive per-page prefix sum (FSM: seed -> steady, SUB_DIM_DONE -> step
# resets the scan state to the current product, same state-machine shape as
# TENSOR_PAGED_MASK).  The page sum lands on the 3rd output element; the two
# prefix elements are steered to junk/stale lanes via the out AP strides.
from concourse.dve_spec import (_State, _Placement, _Stage, _assemble, PREV,
                                Spec, Src0, Src1, Zero, scan, COUNT_ONCE)
from concourse.dve_uop import (AluOp as DAlu, AluInp, Trigger, OutSel,
                               N_STAGES, DveOpSpec)
from concourse import dve_ops as _dve_ops
from concourse.dve_ops import DveOp


def _register_pairsum3():
    name = "LSTM_PAIRSUM3_ANT"
    if name in _dve_ops._SUB_OPCODE_FOR_NAME:
        return next(o for o in _dve_ops.OPS if o.name == name)
    empty = _Stage(DAlu.BYPASS, PREV)
    pipeline = [empty] * N_STAGES["v3"]
    pipeline[0] = _Stage(DAlu.MULTIPLY, Src0, Src1)
    pipeline[1] = _Stage(DAlu.ADD, AluInp.CURR_ALU_OUT, PREV)
    p = _Placement(pipeline=pipeline, node_stage={},
                   lane={Src0: 0, Src1: 1, Zero: 2},
                   out_sel=OutSel.ALU_OUT, accum_stage=None, captures=[])
    states = [
        _State(placement=p, trigger=COUNT_ONCE, repeat=1, next=(1, 0, 0),
               overrides={1: _Stage(DAlu.BYPASS, Zero, Zero)},
               write_out=False),
        _State(placement=p, consume=(True, True),
               trigger=(Trigger.SRC_TENSOR_DONE, Trigger.SUB_DIM_DONE,
                        Trigger.NONE), next=(0, 2, 0)),
        _State(placement=p, consume=(True, True), repeat=1,
               overrides={1: _Stage(DAlu.ADD, Zero, PREV)},
               trigger=(Trigger.SRC_TENSOR_DONE, Trigger.SUB_DIM_DONE,
                        Trigger.COUNT), next=(0, 2, 1)),
    ]
    uops = [_assemble(s) for s in states]
    for u in uops:
        u.validate("v3")
    opcode = max(_dve_ops._SUB_OPCODE_FOR_NAME.values()) + 1
    assert opcode < 0x20
    spec = Spec(body=scan(DAlu.ADD, Src0 * Src1))  # introspection-only
    op = DveOp(name, spec, subdim=True, uops_sha={})
    _dve_ops._COMPILE_CACHE[(name, "v3")] = DveOpSpec(
        name=name, opcode=opcode, uops=uops, rd1_en=True)
    _dve_ops.OPS.append(op)
    _dve_ops.CUSTOM_DVE_SPECS[name] = spec
    _dve_ops._SUB_OPCODE_FOR_NAME[name] = opcode
    return op


PAIRSUM3 = _register_pairsum3()

F32 = mybir.dt.float32
F32R = mybir.dt.float32r
BF16 = mybir.dt.bfloat16
F8 = mybir.dt.float8e4
AF = mybir.ActivationFunctionType

H = 128          # hidden
D = 2048         # input size
B = 64           # batch
T = 256          # seq len
NCORES = 8
BS = B // NCORES          # 8 examples per core
KT = D // 128             # 16 k-tiles of the input GEMM
NCHUNK = 4                # GEMM token chunks
CTOK = T * BS // NCHUNK   # 512 tokens per chunk
TW = CTOK // BS           # 64 timesteps per chunk
W = 8                     # recurrence window (psum burst granularity)
NW = T // W
LAG = 2                   # layer1 runs LAG steps behind layer0
GORD = [0, 1, 2, 3]       # our gate order [i,f,g,o] -> torch block index
P0TOK = [64, 64, 128, 256]  # chunk-0 GEMM pass sizes (tokens)
BLOBW = 1536              # packed-constant blob width (bf16 cols)


def _build_phase2_step(nc, u, P, hwin, hinit, whh0t_s, whh1t_s, wih1t_s,
                       ew, tcT):
    """Emit one macro-step: layer0 step u, layer1 step u-LAG."""
    w, s = divmod(u, W)
    active = []
    if u < T:
        active.append(0)
    if u >= LAG:
        active.append(1)

    def hprev(layer, step):
        if step == 0:
            return hinit[:, layer, :]
        pu = step - 1 + (LAG if layer == 1 else 0)
        return hwin[:, (pu // W) % 2, pu % W, layer, :]

    # layer1 input projection for step v=u-LAG: depends on h0(v), which was
    # produced LAG steps ago -> executes early on PE, off the critical path
    if 1 in active:
        v = u - LAG
        h0v = hwin[:, (v // W) % 2, v % W, 0, :]
        for gi in range(4):
            nc.tensor.matmul(P[:, 1, gi, s, :], wih1t_s[:, gi, :],
                             h0v, start=False, stop=False,
                             skip_group_check=True)

    # step matmuls
    for gi in (0, 1, 2, 3):
        for l in active:
            st = u if l == 0 else u - LAG
            lhs = whh0t_s if l == 0 else whh1t_s
            nc.tensor.matmul(P[:, l, gi, s, :], lhs[:, gi, :], hprev(l, st),
                             start=False, stop=True, skip_group_check=True)

    lo = active[0]
    ln = len(active)
    L = slice(lo, lo + ln)
    # g-gate weights/bias pre-scaled by 2 on host (tanh(a) = 2*sig(2a)-1)
    # and the cell state tracked as C = c/2 (exact), so the update is an
    # EQUAL-weight 3-product sum:  C' = sf*C + sg*si + si*(-0.5).
    # ew lanes (lane-major [128, lane, l, b]) double-buffer the state in A/B
    # blocks so ONE PAIRSUM3 custom op does products+sum in a single Vector
    # instruction: A = [K@0, C@1, si@2, sf@3, sg@4], B = [K@8, C@9, si@10,
    # sf@11, sg@12], so@6.  Even steps read A (in0 = lanes 2:5 overlapping
    # in1 = 0:3, the baseline trick) and the out AP (stride +2 lanes) steers
    # the two prefix elements to junk lanes 5,7 and the page sum to C-B@9;
    # odd steps read B and write (5,3,1) descending, landing C' on C-A@1.
    # Within-op write-then-read hazards: none (writes trail reads per
    # element, and junk targets are never read by the same op).
    A = (u % 2 == 0)
    gb = 2 if A else 10
    nc.scalar.activation(ew[:, gb:gb + 3, L, :],
                         P[:, L, 0:3, s, :].rearrange("p l g b -> p g l b"),
                         AF.Sigmoid)
    nc.scalar.activation(ew[:, 6, L, :], P[:, L, 3, s, :], AF.Sigmoid)
    nc.vector._custom_dve(
        PAIRSUM3,
        out=(ew[:, 5:10:2, L, :] if A
             else ew[:, 5:0:-2, L, :]).rearrange("p k l b -> p (l b) k"),
        in0=ew[:, gb:gb + 3, L, :].rearrange("p k l b -> p (l b) k"),
        in1=ew[:, gb - 2:gb + 1, L, :].rearrange("p k l b -> p (l b) k"))
    cl = 9 if A else 1
    nc.scalar.activation(tcT[:, L, :], ew[:, cl, L, :], AF.Tanh, scale=2.0)
    nc.vector.tensor_mul(hwin[:, w % 2, s, L, :], ew[:, 6, L, :],
                         tcT[:, L, :])


def build_program(debug_taps=False):
    nc = bacc.Bacc("TRN2", target_bir_lowering=False, debug=False,
                   enable_asserts=False)

    # ---- DRAM I/O ----
    xd = nc.dram_tensor("xp", [NCHUNK, 128, KT, CTOK], F32R,
                        kind="ExternalInput").ap()
    # all bulk tensors are flat [128, X] so their DMA APs have a large
    # innermost dim (the cost model's descriptor unit is the lowest AP dim)
    wih0d = nc.dram_tensor("wih0t", [128, KT * 512], F32R,
                           kind="ExternalInput").ap()
    # pass-0 x (first KT*64 cols) and the fp8 chunk-0 weights share one
    # dram tensor so the first startup DMA covers both (saves a HWDGE slot)
    wx0f8d = nc.dram_tensor("wx0f8", [128, KT * 64 + KT * 512], F8,
                            kind="ExternalInput").ap()
    xq_d = [None] + [nc.dram_tensor(f"x0q{i}", [128, KT * P0TOK[i]], F8,
                                    kind="ExternalInput").ap()
                     for i in range(1, len(P0TOK))]
    # weight blob (cols 0:512 whh0t, 512:1024 whh1t, 1024:1536 wih1t) is
    # DMA'd in two slices so whh0t lands early; every constant the first
    # recurrence window needs rides in the tiny cbias DMA (parts 0:4:
    # b0s4@0:128, onehot@128:384, b1s@1408:1536; part 0: b0s@384:896,
    # ones@896:1408) -- b1s must NOT wait on the blob, the u=0 layer-1
    # onehot matmul queues ahead of u=0's step matmuls on the PE.
    blobd = nc.dram_tensor("cblob", [128, BLOBW], BF16,
                           kind="ExternalInput").ap()
    cbiasd = nc.dram_tensor("cbias", [4, 1536], BF16,
                            kind="ExternalInput").ap()
    identd = nc.dram_tensor("ident", [128, 128], F32,
                            kind="ExternalInput").ap()
    wfcd = nc.dram_tensor("wfct", [128, 1], BF16, kind="ExternalInput").ap()
    bfcd = nc.dram_tensor("bfcb", [1, BS], F32, kind="ExternalInput").ap()
    yd = nc.dram_tensor("y", [BS, 1], F32, kind="ExternalOutput").ap()
    if debug_taps:
        dbg_xp0 = nc.dram_tensor("dbg_xp0", [128, 4, TW, BS], F32,
                                 kind="ExternalOutput").ap()
        dbg_hwin = nc.dram_tensor("dbg_hwin", [128, 2, W, 2, BS], BF16,
                                  kind="ExternalOutput").ap()
        dbg_c = nc.dram_tensor("dbg_c", [128, 2, BS], F32,
                               kind="ExternalOutput").ap()

    with tile.TileContext(nc) as tc, \
            tc.tile_pool(name="persist", bufs=1) as pp:
        # ---- persistent SBUF (bulk tensors flat, views for compute) ----
        wih0t_s = pp.tile([128, KT * 512], F32R, name="wih0t_s")
        wx0f8_s = pp.tile([128, KT * 64 + KT * 512], F8, name="wx0f8_s")
        wih0f8_s = wx0f8_s[:, KT * 64:]
        xq_s = [wx0f8_s[:, 0:KT * 64]] + [
            pp.tile([128, KT * n], F8, name=f"xq{i}_s")
            for i, n in enumerate(P0TOK) if i >= 1]
        blob_s = pp.tile([128, BLOBW], BF16, name="blob_s")
        cbias_s = pp.tile([4, 1536], BF16, name="cbias_s")
        ident_s = pp.tile([128, 128], F32, name="ident_s")
        wfct_s = pp.tile([128, 1], BF16, name="wfct_s")
        bfcb_s = pp.tile([1, BS], F32, name="bfcb_s")
        wones = pp.tile([1, 256], BF16, name="wones")
        y_sb = pp.tile([BS, 1], F32, name="y_sb")

        def wsl(wtile, k, g):  # [K=128, 128] lhsT slice of a flat w tile
            o = (k * 4 + g) * 128
            return wtile[:, o:o + 128]

        # constant views into the blob
        whh0t_s = blob_s[:, 0:512].rearrange("p (g u) -> p g u", g=4)
        whh1t_s = blob_s[:, 512:1024].rearrange("p (g u) -> p g u", g=4)
        wih1t_s = blob_s[:, 1024:1536].rearrange("p (g u) -> p g u", g=4)
        b1s_s = cbias_s[0:4, 1408:1536]
        b0s4_s = cbias_s[0:4, 0:128]
        onehot_s = cbias_s[0:4, 128:384].rearrange("p (g n) -> p g n", g=4)
        b0s_s = cbias_s[0:1, 384:896].rearrange("p (g u) -> p g u", g=4)
        ones_s = cbias_s[0:1, 896:1408]

        # Startup DMA schedule.  HWDGE generation is a single shared
        # serialized resource (~630ns/DMA) and a DMA instruction occupies
        # its queue's SEQ until HWDGE accepts it, so: recurrence-critical
        # DMAs first on sync in priority order, bulky weights on gpsimd
        # (SWDGE -- separate generator, keeps HWDGE free), and NOTHING on
        # the scalar/vector queues (their SEQs feed the recurrence).
        # Critical set for the first window: blob, wih0f8, xq0, ident.
        # single-queue DMA schedule in priority order (the DMA_ENGINES
        # transfer order is FIFO by generation-completion, so sync-queue
        # issue order IS the arrival order); chunk 0 uses the fp8 weights
        # throughout -- its xp0 error decays through >190 forget gates
        nc.sync.dma_start(cbias_s[:], cbiasd[:])
        nc.sync.dma_start(wx0f8_s[:, 0:3072], wx0f8d[:, 0:3072])
        for q in range(1, 4):
            o = KT * 64 + q * 2048
            nc.sync.dma_start(wx0f8_s[:, o:o + 2048], wx0f8d[:, o:o + 2048])
        nc.sync.dma_start(blob_s[:, 0:512], blobd[:, 0:512])
        nc.sync.dma_start(blob_s[:, 512:BLOBW], blobd[:, 512:BLOBW])
        nc.sync.dma_start(xq_s[1][:], xq_d[1][:])
        nc.sync.dma_start(ident_s[:], identd[:])
        nc.sync.dma_start(xq_s[2][:], xq_d[2][:])
        nc.sync.dma_start(xq_s[3][:], xq_d[3][:])
        nc.sync.dma_start(wfct_s[:], wfcd[:])
        nc.sync.dma_start(bfcb_s[:], bfcd[:])
        nc.sync.dma_start(wih0t_s[:], wih0d[:])

        # xp0 per chunk: [128, gate, t-local, b] fp32
        xp0_t = [pp.tile([128, 4, TW, BS], F32, name=f"xp0_{c}")
                 for c in range(NCHUNK)]

        # recurrence state: ew lane-major [128, lane, l, b]; lanes
        # [0:K-A, 1:C-A, 2:si-A, 3:sf-A, 4:sg-A, 5:junk, 6:so, 7:junk,
        #  8:K-B, 9:C-B, 10:si-B, 11:sf-B, 12:sg-B, 13:spare]
        hinit = pp.tile([128, 2, BS], BF16, name="hinit")
        hwin = pp.tile([128, 2, W, 2, BS], BF16, name="hwin")
        ew = pp.tile([128, 14, 2, BS], F32, name="ew")
        tcT = pp.tile([128, 2, BS], F32, name="tcT")

        nc.vector.memset(ew[:], 0.0)
        nc.vector.memset(ew[:, 0, :, :], -0.5)
        nc.vector.memset(ew[:, 8, :, :], -0.5)
        nc.vector.memset(hinit[:], 0.0)
        nc.vector.memset(wones[:], 0.0)
        # pre-warm the ACT function table (LoadActFuncSet ~1.3us) off-chain
        nc.scalar.activation(tcT[:, 0:1, :], ew[:, 1, 0:1, :], AF.Sigmoid)
        nc.scalar.activation(tcT[:, 0:1, :], ew[:, 1, 0:1, :], AF.Tanh)

        with (
            tc.tile_pool(name="xchunk", bufs=2) as x_pool,
            tc.tile_pool(name="gemm_ps", bufs=4, space="PSUM") as gemm_ps,
            tc.tile_pool(name="pair_ps", bufs=2, space="PSUM") as pair_ps,
        ):
            # ---- one chunk-0 GEMM pass: bias + k-MMs + copies ----
            def emit_pass_ops(i):
                ntok = P0TOK[i]
                toff = sum(P0TOK[:i])
                t0 = toff // BS
                tw_ = ntok // BS
                pg = []
                for g in range(4):
                    p = gemm_ps.tile([128, CTOK], F32, name="pg")
                    pg.append(p)
                    nc.tensor.matmul(p[:, 0:ntok], b0s_s[:, g, :],
                                     ones_s[:, 0:ntok],
                                     start=True, stop=False,
                                     skip_group_check=True)
                    yield 1
                for k in range(KT):
                    for g in range(4):
                        nc.tensor.matmul(
                            pg[g][:, 0:ntok], wsl(wih0f8_s, k, g),
                            xq_s[i][:, k * ntok:(k + 1) * ntok],
                            start=False, stop=(k == KT - 1),
                            skip_group_check=True)
                        yield 1
                for g in range(4):
                    dst = xp0_t[0][:, g, t0:t0 + tw_, :]
                    srcv = pg[g][:, 0:ntok].rearrange(
                        "p (t b) -> p t b", t=tw_)
                    nc.scalar.copy(dst, srcv)
                    yield 1

            # ---- GEMM op generator (pulled incrementally) ----
            def gemm_gen():
                # chunk-0 passes 1..3 (pass 0 runs in the prologue); x was
                # DMA'd up front, so only PE/copy ops are paced here
                for i in range(1, len(P0TOK)):
                    yield from emit_pass_ops(i)
                # chunks 1-3: f32r, per-k slab DMAs pace the matmuls
                for c in range(1, NCHUNK):
                    xt = x_pool.tile([128, KT, CTOK], F32R, name="xt")
                    for k in range(KT):
                        nc.sync.dma_start(xt[:, k, :], xd[c, :, k, :])
                        yield 1
                    # 256-col halves: same f32r rate (full at N>=256) but a
                    # straddling matmul blocks the chain MMs in the in-order
                    # PE queue for at most ~107ns instead of ~213ns
                    pg = []
                    for g in range(4):
                        p = gemm_ps.tile([128, CTOK], F32, name="pg")
                        pg.append(p)
                        for hh in range(2):
                            cs = slice(hh * 256, (hh + 1) * 256)
                            nc.tensor.matmul(p[:, cs], b0s_s[:, g, :],
                                             ones_s[:, cs],
                                             start=True, stop=False,
                                             skip_group_check=True)
                            yield 1
                    for k in range(KT):
                        for g in range(4):
                            for hh in range(2):
                                cs = slice(hh * 256, (hh + 1) * 256)
                                nc.tensor.matmul(
                                    pg[g][:, cs], wsl(wih0t_s, k, g),
                                    xt[:, k, cs],
                                    start=False, stop=(k == KT - 1),
                                    skip_group_check=True)
                                yield 1
                    for g in range(4):
                        for hh in range(2):
                            tsl = slice(hh * TW // 2, (hh + 1) * TW // 2)
                            dst = xp0_t[c][:, g, tsl, :]
                            srcv = pg[g][:, hh * 256:(hh + 1) * 256].rearrange(
                                "p (t b) -> p t b", t=TW // 2)
                            nc.scalar.copy(dst, srcv)
                            yield 1

            gen = gemm_gen()

            def pull(n):
                for _ in range(n):
                    if next(gen, None) is None:
                        break

            # ---- prologue ----
            # PE warm-up: ~4us of dummy matmuls ramp the PE out of its cold
            # p-state so pass 0 runs at full clock the moment wih0f8 lands
            warm = gemm_ps.tile([128, CTOK], F32, name="pg")
            for _ in range(16):
                nc.tensor.matmul(warm[:, 0:256], wones[0:1, 0:128],
                                 wones[:, 0:256], start=True, stop=True,
                                 skip_group_check=True)
            # pass 0 (fp8, first window) accumulates STRAIGHT INTO the
            # window-0 PSUM pair: no xp0 store, no copies, no identity
            # injection on the critical path.  Layer 0's bias lands via the
            # same K=4 onehot trick as layer 1's (ONE start=True per bank).
            P0 = pair_ps.tile([128, 2, 4, 16, BS], F32, name="pairP")
            nc.tensor.matmul(P0[:, 0, :, 0:W, :], b0s4_s[:, :],
                             onehot_s[:, :, :],
                             start=True, stop=False, skip_group_check=True)
            for k in range(KT):
                for g in range(4):
                    nc.tensor.matmul(
                        P0[:, 0, g, 0:W, :].rearrange("p s b -> p (s b)"),
                        wsl(wih0f8_s, k, g),
                        xq_s[0][:, k * W * BS:(k + 1) * W * BS],
                        start=False, stop=False, skip_group_check=True)

            P = None
            for u in range(T + LAG):
                w, s = divmod(u, W)
                if s == 0:
                    P = P0 if u == 0 else pair_ps.tile(
                        [128, 2, 4, 16, BS], F32, name="pairP")
                    if u < T and u > 0:
                        c, lw = divmod(w, TW // W)
                        nc.tensor.matmul(
                            P[:, 0, :, 0:W, :],
                            ident_s[:, :],
                            xp0_t[c][:, :, lw * W:(lw + 1) * W, :],
                            start=True, stop=False, skip_group_check=True)
                    if u + W > LAG:
                        # whole-bank bias broadcast in ONE start=True matmul
                        # (start=True clears has_written for the full bank);
                        # Wih1 @ h0 is added per-step (off the critical path).
                        nc.tensor.matmul(
                            P[:, 1, :, 0:W, :], b1s_s[:, :], onehot_s[:, :, :],
                            start=True, stop=False, skip_group_check=True)
                _build_phase2_step(nc, u, P, hwin, hinit, whh0t_s, whh1t_s,
                                   wih1t_s, ew, tcT)
                # GEMM-op interleave AFTER the step's chain matmuls: the
                # in-order PE exec queue then holds [chain MMs (sem-gated),
                # GEMM MMs], so GEMM work fills the elementwise-phase idle
                # window and never straddles the h-sem release (straddling
                # 213ns f32r matmuls cost ~4us of stragglers otherwise).
                # Start at u>=4 so the queue never stalls on a matmul whose
                # x DMA is still in flight; 8/step drains chunk-0 passes
                # 1-3 early enough for their windows while chunks 1-3 pace
                # on their slab DMAs.
                if u >= 4:
                    pull(8 if u < 32 else 6 if u < 96 else 4)
                if debug_taps and u == 31:
                    nc.sync.dma_start(dbg_xp0[:], xp0_t[0][:])
                    nc.sync.dma_start(dbg_hwin[:], hwin[:])
                    nc.sync.dma_start(dbg_c[:], cC[:])

            pull(10000)  # drain any leftovers (shouldn't be needed)

            # ---- final fc: bias folded in as a K=1 matmul (ident[0,0]
            # supplies the f32 one), y DMA'd straight from PSUM ----
            fcp = gemm_ps.tile([BS, 1], F32, name="pg")
            nc.tensor.matmul(fcp[:, :], hwin[:, (T + LAG - 1) // W % 2,
                                             (T + LAG - 1) % W, 1, :],
                             wfct_s[:, :], start=True, stop=False,
                             skip_group_check=True)
            nc.tensor.matmul(fcp[:, :], bfcb_s[:, :], ident_s[0:1, 0:1],
                             start=False, stop=True, skip_group_check=True)
            nc.vector.tensor_copy(y_sb[:, :], fcp[:, :])
            nc.sync.dma_start(yd[:], y_sb[:])

    nc.compile()
    return nc


_PROG = None


def _get_program():
    global _PROG
    if _PROG is None:
        _PROG = build_program()
    return _PROG


def prep_inputs(x, Wih0, Whh0, bih0, bhh0, Wih1, Whh1, bih1, bhh1, Wfc, bfc):
    """Host-side layout prep -> per-core in_maps."""
    bf = ml_dtypes.bfloat16
    f8 = ml_dtypes.float8_e4m3
    x = np.asarray(x, np.float32)

    # weights: [4H, K] -> [K(part), gate(ours), unit]
    def gate_T(Wmat):  # [512, K] -> [K, 4, 128] in our gate order
        A = np.asarray(Wmat, np.float32).reshape(4, 128, -1)  # tg, j, k
        A = A.transpose(2, 0, 1)[:, GORD, :]                  # k, ours, j
        A = A.copy()
        A[:, 2, :] *= 2.0  # tanh-gate folded 2x (tanh(a)=2*sig(2a)-1)
        return np.ascontiguousarray(A)

    wih0t = gate_T(Wih0).reshape(KT, 128, 4, 128).transpose(1, 0, 2, 3)
    wih0t = np.ascontiguousarray(wih0t, np.float32)           # [128,KT,4,128]
    whh0t = gate_T(Whh0).astype(bf)                           # [128,4,128]
    whh1t = gate_T(Whh1).astype(bf)
    wih1t = gate_T(Wih1).astype(bf)

    b0 = (np.asarray(bih0) + np.asarray(bhh0)).astype(np.float32)
    b1 = (np.asarray(bih1) + np.asarray(bhh1)).astype(np.float32)
    b0s = b0.reshape(4, 128)[GORD].copy()
    b0s[2] *= 2.0                                             # [4,128]
    b1g4 = b1.reshape(4, 128)[GORD].copy()
    b1g4[2] *= 2.0                                            # [4,128]
    ident = np.eye(128, dtype=np.float32)
    wfct = np.asarray(Wfc, np.float32).T.astype(bf)           # [128,1]
    bfcb = np.full((1, BS), np.asarray(bfc, np.float32)[0], np.float32)

    # packed constant blob (single startup DMA); layout must match the
    # blob_s views in build_program
    blob = np.zeros((128, BLOBW), np.float32)
    blob[:, 0:512] = whh0t.reshape(128, 512)
    blob[:, 512:1024] = whh1t.reshape(128, 512)
    blob[:, 1024:1536] = wih1t.reshape(128, 512)
    blob = blob.astype(bf)
    cbias = np.zeros((4, 1536), np.float32)
    cbias[0:4, 0:128] = b0s
    cbias[0:4, 128:384] = np.repeat(np.eye(4, dtype=np.float32),
                                    W * BS).reshape(4, 4 * W * BS)
    cbias[0, 384:896] = b0s.reshape(512)
    cbias[0, 896:1408] = 1.0
    cbias[0:4, 1408:1536] = b1g4
    cbias = cbias.astype(bf)

    wih0flat = wih0t.reshape(128, KT * 512)
    common = dict(wih0t=wih0flat,
                  cblob=blob, cbias=cbias, ident=ident, wfct=wfct,
                  bfcb=bfcb)

    offs = np.cumsum([0] + P0TOK)
    in_maps = []
    for c in range(NCORES):
        xs = x[c * BS:(c + 1) * BS]                           # [BS, T, D]
        xt = xs.transpose(2, 1, 0).reshape(D, T * BS)         # [d, tok(t,b)]
        xpre = (xt.reshape(KT, 128, NCHUNK, CTOK)
                .transpose(2, 1, 0, 3))                       # [c,128,k,tok]
        m = {"xp": np.ascontiguousarray(xpre, np.float32), **common}
        for i in range(len(P0TOK)):
            seg = np.ascontiguousarray(xpre[0][:, :, offs[i]:offs[i + 1]])
            seg = seg.astype(f8).reshape(128, KT * P0TOK[i])
            if i == 0:
                m["wx0f8"] = np.concatenate(
                    [seg, wih0flat.astype(f8)], axis=1)
            else:
                m[f"x0q{i}"] = seg
        in_maps.append(m)
    return in_maps


def run(inputs, **kw):
    nc = _get_program()
    in_maps = prep_inputs(**inputs)
    res = run_bass_kernel_spmd(nc, in_maps, core_ids=list(range(NCORES)), **kw)
    y = np.concatenate([res.results[c]["y"] for c in range(NCORES)], axis=0)
    return y.astype(np.float32), res


def kernel(**inputs):
    y, _ = run(inputs)
    return y


if __name__ == "__main__":
    import sys
    if "--sim" in sys.argv:
        import trails.perfetto as _tp
        if not hasattr(_tp.LazyPerfetto, "add_counter"):
            def _add_counter(self, proc, track, ts_, val):
                self.update_counter(proc, track, int(ts_), float(val),
                                    unit="ns")
            _tp.LazyPerfetto.add_counter = _add_counter
        for _m in ("enable_explicit_ordering", "reserve_process_order"):
            if not hasattr(_tp.LazyPerfetto, _m):
                setattr(_tp.LazyPerfetto, _m,
                        lambda self, *a, **k: None)
        from concourse.timeline_sim import TimelineSim
        nc = _get_program()
        ts = TimelineSim(nc, trace="--trace" in sys.argv)
        dur = ts.simulate()
        print(f"TimelineSim predicted duration: {dur:.0f} ns")
        if ts.perfetto is not None:
            ts.perfetto.save("/root/problem/timeline.pftrace")
            print("wrote /root/problem/timeline.pftrace")



# revision 20
# speedup vs baseline: 1.0223x; 1.0222x over previous
"""Trainium2 Bass kernel for a 2-layer LSTM binary classifier.

Model: xp0 = x @ Wih0.T + b0 ; layer0 LSTM ; xp1 = seq0 @ Wih1.T + b1 ;
layer1 LSTM ; out = h1_T @ Wfc.T + bfc.

Sharding: data-parallel over batch (64 -> 8 cores x 8 examples), all
weights replicated.  Per core:
  Phase 1 (interleaved with phase 2): big input GEMM, bias added via
    ones/onehot matmuls, output xp0 stored in SBUF as [128, gate, t, b].
    Chunk 0 (t<64) runs on fp8e4 weights (its xp0 error decays through
    >190 forget gates); later chunks use float32r (full PE rate at
    N>=256, ~tf32).  Startup is latency-tuned: HWDGE generation is ONE
    shared serialized resource (~630ns/DMA) and a DMA occupies its
    queue's SEQ until HWDGE accepts it, so all DMAs sit on the sync
    queue in priority order, constants ride in one packed blob, bulk
    tensors are flat [128,X] (the DMA cost model's descriptor unit is
    the lowest AP dim), ~4us of dummy matmuls pre-ramp the PE p-state,
    and pass 0 of the GEMM accumulates straight into the window-0 PSUM
    pair (layer-0 bias via the same K=4 onehot trick as layer 1) so the
    first recurrence step issues ~7.6us after kernel start.
  Phase 2: serial recurrence, the wall-clock driver (~1.79us/step chain
    latency x 258 macro-steps).  Gates live as [gate-dim on partitions,
    batch on free].  Per macro-step u: layer0 runs step u and layer1 runs
    step u-LAG so both layers share joint elementwise instructions.
    xp contributions are pre-accumulated into PSUM per W-step window
    (identity matmul for layer0's xp0; a single K=4 gate-onehot matmul
    broadcasts layer1's bias and must be the only start=True write to the
    bank -- start=True clears the whole bank's has_written bits).  Wih1 @
    h0(v) runs as per-step matmuls that depend on LAG-old data, so they
    execute off the critical path.  tanh-gate weights are pre-scaled 2x on
    the host so sigmoid covers the g-gate too (tanh(a) = 2*sig(2a)-1);
    the o-gate's sigmoid is a separate ACT op because it is only needed
    at the chain tail, so the on-chain sigmoid(i,f,g) is gated by just 6
    of 8 matmuls.  The 3-product cell update (tensor_mul + tensor_reduce
    originally) is ONE hand-built custom DVE op (LSTM_PAIRSUM3_ANT): a
    segmented product-scan over pages of 3 whose FSM (seed/steady/step,
    SUB_DIM_DONE reset -- the TENSOR_PAGED_MASK state-machine shape)
    resets the prefix sum at page boundaries; the out AP steers the two
    prefix elements onto junk/stale lanes and the page sum onto the
    opposite A/B state block's C lane.  Per-step chain: 6 bf16 matmuls ->
    sigmoid_ifg (ACT) -> PAIRSUM3 (DVE) -> tanh (ACT) -> h-mul (DVE,
    bf16 out) -> next step's matmuls; ~1.57us/step (was ~1.79us), every
    link at the cost model's per-op floor (ACT ops carry ~370ns of SBUF
    access latency, DVE ~120ns, PE matmuls a fixed 173ns drain).
"""

import numpy as np
import ml_dtypes

import concourse.bass as bass
import concourse.tile as tile
from concourse import bacc, mybir
from concourse.bass_utils import run_bass_kernel_spmd

# ---- custom DVE op: segmented product-sum (pages of 3) -------------------
# One Vector instruction replacing [tensor_mul(3 lanes) + tensor_reduce]:
# streams in0/in1 as [P, S, N=3] pages; per element computes p = in0*in1 and
# an inclusive per-page prefix sum (FSM: seed -> steady, SUB_DIM_DONE -> step
# resets the scan state to the current product, same state-machine shape as
# TENSOR_PAGED_MASK).  The page sum lands on the 3rd output element; the two
# prefix elements are steered to junk/stale lanes via the out AP strides.
from concourse.dve_spec import (_State, _Placement, _Stage, _assemble, PREV,
                                Spec, Src0, Src1, Zero, scan, COUNT_ONCE)
from concourse.dve_uop import (AluOp as DAlu, AluInp, Trigger, OutSel,
                               N_STAGES, DveOpSpec)
from concourse import dve_ops as _dve_ops
from concourse.dve_ops import DveOp


def _register_pairsum3():
    name = "LSTM_PAIRSUM3_ANT"
    if name in _dve_ops._SUB_OPCODE_FOR_NAME:
        return next(o for o in _dve_ops.OPS if o.name == name)
    empty = _Stage(DAlu.BYPASS, PREV)
    pipeline = [empty] * N_STAGES["v3"]
    pipeline[0] = _Stage(DAlu.MULTIPLY, Src0, Src1)
    pipeline[1] = _Stage(DAlu.ADD, AluInp.CURR_ALU_OUT, PREV)
    p = _Placement(pipeline=pipeline, node_stage={},
                   lane={Src0: 0, Src1: 1, Zero: 2},
                   out_sel=OutSel.ALU_OUT, accum_stage=None, captures=[])
    states = [
        _State(placement=p, trigger=COUNT_ONCE, repeat=1, next=(1, 0, 0),
               overrides={1: _Stage(DAlu.BYPASS, Zero, Zero)},
               write_out=False),
        _State(placement=p, consume=(True, True),
               trigger=(Trigger.SRC_TENSOR_DONE, Trigger.SUB_DIM_DONE,
                        Trigger.NONE), next=(0, 2, 0)),
        _State(placement=p, consume=(True, True), repeat=1,
               overrides={1: _Stage(DAlu.ADD, Zero, PREV)},
               trigger=(Trigger.SRC_TENSOR_DONE, Trigger.SUB_DIM_DONE,
                        Trigger.COUNT), next=(0, 2, 1)),
    ]
    uops = [_assemble(s) for s in states]
    for u in uops:
        u.validate("v3")
    opcode = max(_dve_ops._SUB_OPCODE_FOR_NAME.values()) + 1
    assert opcode < 0x20
    spec = Spec(body=scan(DAlu.ADD, Src0 * Src1))  # introspection-only
    op = DveOp(name, spec, subdim=True, uops_sha={})
    _dve_ops._COMPILE_CACHE[(name, "v3")] = DveOpSpec(
        name=name, opcode=opcode, uops=uops, rd1_en=True)
    _dve_ops.OPS.append(op)
    _dve_ops.CUSTOM_DVE_SPECS[name] = spec
    _dve_ops._SUB_OPCODE_FOR_NAME[name] = opcode
    return op


PAIRSUM3 = _register_pairsum3()

F32 = mybir.dt.float32
F32R = mybir.dt.float32r
BF16 = mybir.dt.bfloat16
F8 = mybir.dt.float8e4
AF = mybir.ActivationFunctionType

H = 128          # hidden
D = 2048         # input size
B = 64           # batch
T = 256          # seq len
NCORES = 8
BS = B // NCORES          # 8 examples per core
KT = D // 128             # 16 k-tiles of the input GEMM
NCHUNK = 4                # GEMM token chunks
CTOK = T * BS // NCHUNK   # 512 tokens per chunk
TW = CTOK // BS           # 64 timesteps per chunk
W = 8                     # recurrence window (psum burst granularity)
NW = T // W
LAG = 2                   # layer1 runs LAG steps behind layer0
GORD = [0, 1, 2, 3]       # our gate order [i,f,g,o] -> torch block index
P0TOK = [64, 64, 128, 256]  # chunk-0 GEMM pass sizes (tokens)
BLOBW = 1536              # packed-constant blob width (bf16 cols)


def _build_phase2_step(nc, u, P, hwin, hinit, whh0t_s, whh1t_s, wih1t_s,
                       ew, tcT):
    """Emit one macro-step: layer0 step u, layer1 step u-LAG."""
    w, s = divmod(u, W)
    active = []
    if u < T:
        active.append(0)
    if u >= LAG:
        active.append(1)

    def hprev(layer, step):
        if step == 0:
            return hinit[:, layer, :]
        pu = step - 1 + (LAG if layer == 1 else 0)
        return hwin[:, (pu // W) % 2, pu % W, layer, :]

    # layer1 input projection for step v=u-LAG: depends on h0(v), which was
    # produced LAG steps ago -> executes early on PE, off the critical path
    if 1 in active:
        v = u - LAG
        h0v = hwin[:, (v // W) % 2, v % W, 0, :]
        for gi in range(4):
            nc.tensor.matmul(P[:, 1, gi, s, :], wih1t_s[:, gi, :],
                             h0v, start=False, stop=False,
                             skip_group_check=True)

    # step matmuls
    for gi in (0, 1, 2, 3):
        for l in active:
            st = u if l == 0 else u - LAG
            lhs = whh0t_s if l == 0 else whh1t_s
            nc.tensor.matmul(P[:, l, gi, s, :], lhs[:, gi, :], hprev(l, st),
                             start=False, stop=True, skip_group_check=True)

    lo = active[0]
    ln = len(active)
    L = slice(lo, lo + ln)
    # g-gate weights/bias pre-scaled by 2 on host (tanh(a) = 2*sig(2a)-1)
    # and the cell state tracked as C = c/2 (exact), so the update is an
    # EQUAL-weight 3-product sum:  C' = sf*C + sg*si + si*(-0.5).
    # ew lanes (lane-major [128, lane, l, b]) double-buffer the state in A/B
    # blocks so ONE PAIRSUM3 custom op does products+sum in a single Vector
    # instruction: A = [K@0, C@1, si@2, sf@3, sg@4], B = [K@8, C@9, si@10,
    # sf@11, sg@12], so@6.  Even steps read A (in0 = lanes 2:5 overlapping
    # in1 = 0:3, the baseline trick) and the out AP (stride +2 lanes) steers
    # the two prefix elements to junk lanes 5,7 and the page sum to C-B@9;
    # odd steps read B and write (5,3,1) descending, landing C' on C-A@1.
    # Within-op write-then-read hazards: none (writes trail reads per
    # element, and junk targets are never read by the same op).
    A = (u % 2 == 0)
    gb = 2 if A else 10
    nc.scalar.activation(ew[:, gb:gb + 3, L, :],
                         P[:, L, 0:3, s, :].rearrange("p l g b -> p g l b"),
                         AF.Sigmoid)
    nc.scalar.activation(ew[:, 6, L, :], P[:, L, 3, s, :], AF.Sigmoid)
    nc.vector._custom_dve(
        PAIRSUM3,
        out=(ew[:, 5:10:2, L, :] if A
             else ew[:, 5:0:-2, L, :]).rearrange("p k l b -> p (l b) k"),
        in0=ew[:, gb:gb + 3, L, :].rearrange("p k l b -> p (l b) k"),
        in1=ew[:, gb - 2:gb + 1, L, :].rearrange("p k l b -> p (l b) k"))
    cl = 9 if A else 1
    nc.scalar.activation(tcT[:, L, :], ew[:, cl, L, :], AF.Tanh, scale=2.0)
    nc.vector.tensor_mul(hwin[:, w % 2, s, L, :], ew[:, 6, L, :],
                         tcT[:, L, :])


def build_program(debug_taps=False):
    nc = bacc.Bacc("TRN2", target_bir_lowering=False, debug=False,
                   enable_asserts=False)

    # ---- DRAM I/O ----
    xd = nc.dram_tensor("xp", [NCHUNK, 128, KT, CTOK], F32R,
                        kind="ExternalInput").ap()
    # all bulk tensors are flat [128, X] so their DMA APs have a large
    # innermost dim (the cost model's descriptor unit is the lowest AP dim)
    wih0d = nc.dram_tensor("wih0t", [128, KT * 512], F32R,
                           kind="ExternalInput").ap()
    # pass-0 x (first KT*64 cols) and the fp8 chunk-0 weights share one
    # dram tensor so the first startup DMA covers both (saves a HWDGE slot)
    wx0f8d = nc.dram_tensor("wx0f8", [128, KT * 64 + KT * 512], F8,
                            kind="ExternalInput").ap()
    xq_d = [None] + [nc.dram_tensor(f"x0q{i}", [128, KT * P0TOK[i]], F8,
                                    kind="ExternalInput").ap()
                     for i in range(1, len(P0TOK))]
    # weight blob (cols 0:512 whh0t, 512:1024 whh1t, 1024:1536 wih1t) is
    # DMA'd in two slices so whh0t lands early; every constant the first
    # recurrence window needs rides in the tiny cbias DMA (parts 0:4:
    # b0s4@0:128, onehot@128:384, b1s@1408:1536; part 0: b0s@384:896,
    # ones@896:1408) -- b1s must NOT wait on the blob, the u=0 layer-1
    # onehot matmul queues ahead of u=0's step matmuls on the PE.
    blobd = nc.dram_tensor("cblob", [128, BLOBW], BF16,
                           kind="ExternalInput").ap()
    cbiasd = nc.dram_tensor("cbias", [4, 1536], BF16,
                            kind="ExternalInput").ap()
    identd = nc.dram_tensor("ident", [128, 128], F32,
                            kind="ExternalInput").ap()
    wfcd = nc.dram_tensor("wfct", [128, 1], BF16, kind="ExternalInput").ap()
    bfcd = nc.dram_tensor("bfcb", [1, BS], F32, kind="ExternalInput").ap()
    yd = nc.dram_tensor("y", [BS, 1], F32, kind="ExternalOutput").ap()
    if debug_taps:
        dbg_xp0 = nc.dram_tensor("dbg_xp0", [128, 4, TW, BS], F32,
                                 kind="ExternalOutput").ap()
        dbg_hwin = nc.dram_tensor("dbg_hwin", [128, 2, W, 2, BS], BF16,
                                  kind="ExternalOutput").ap()
        dbg_c = nc.dram_tensor("dbg_c", [128, 2, BS], F32,
                               kind="ExternalOutput").ap()

    with tile.TileContext(nc) as tc, \
            tc.tile_pool(name="persist", bufs=1) as pp:
        # ---- persistent SBUF (bulk tensors flat, views for compute) ----
        wih0t_s = pp.tile([128, KT * 512], F32R, name="wih0t_s")
        wx0f8_s = pp.tile([128, KT * 64 + KT * 512], F8, name="wx0f8_s")
        wih0f8_s = wx0f8_s[:, KT * 64:]
        xq_s = [wx0f8_s[:, 0:KT * 64]] + [
            pp.tile([128, KT * n], F8, name=f"xq{i}_s")
            for i, n in enumerate(P0TOK) if i >= 1]
        blob_s = pp.tile([128, BLOBW], BF16, name="blob_s")
        cbias_s = pp.tile([4, 1536], BF16, name="cbias_s")
        ident_s = pp.tile([128, 128], F32, name="ident_s")
        wfct_s = pp.tile([128, 1], BF16, name="wfct_s")
        bfcb_s = pp.tile([1, BS], F32, name="bfcb_s")
        wones = pp.tile([1, 256], BF16, name="wones")
        y_sb = pp.tile([BS, 1], F32, name="y_sb")

        def wsl(wtile, k, g):  # [K=128, 128] lhsT slice of a flat w tile
            o = (k * 4 + g) * 128
            return wtile[:, o:o + 128]

        # constant views into the blob
        whh0t_s = blob_s[:, 0:512].rearrange("p (g u) -> p g u", g=4)
        whh1t_s = blob_s[:, 512:1024].rearrange("p (g u) -> p g u", g=4)
        wih1t_s = blob_s[:, 1024:1536].rearrange("p (g u) -> p g u", g=4)
        b1s_s = cbias_s[0:4, 1408:1536]
        b0s4_s = cbias_s[0:4, 0:128]
        onehot_s = cbias_s[0:4, 128:384].rearrange("p (g n) -> p g n", g=4)
        b0s_s = cbias_s[0:1, 384:896].rearrange("p (g u) -> p g u", g=4)
        ones_s = cbias_s[0:1, 896:1408]

        # Startup DMA schedule.  HWDGE generation is a single shared
        # serialized resource (~630ns/DMA) and a DMA instruction occupies
        # its queue's SEQ until HWDGE accepts it, so: recurrence-critical
        # DMAs first on sync in priority order, bulky weights on gpsimd
        # (SWDGE -- separate generator, keeps HWDGE free), and NOTHING on
        # the scalar/vector queues (their SEQs feed the recurrence).
        # Critical set for the first window: blob, wih0f8, xq0, ident.
        # single-queue DMA schedule in priority order (the DMA_ENGINES
        # transfer order is FIFO by generation-completion, so sync-queue
        # issue order IS the arrival order); chunk 0 uses the fp8 weights
        # throughout -- its xp0 error decays through >190 forget gates
        nc.sync.dma_start(cbias_s[:], cbiasd[:])
        nc.sync.dma_start(wx0f8_s[:, 0:3072], wx0f8d[:, 0:3072])
        for q in range(1, 4):
            o = KT * 64 + q * 2048
            nc.sync.dma_start(wx0f8_s[:, o:o + 2048], wx0f8d[:, o:o + 2048])
        nc.sync.dma_start(blob_s[:, 0:512], blobd[:, 0:512])
        nc.sync.dma_start(blob_s[:, 512:BLOBW], blobd[:, 512:BLOBW])
        nc.sync.dma_start(xq_s[1][:], xq_d[1][:])
        nc.sync.dma_start(ident_s[:], identd[:])
        nc.sync.dma_start(xq_s[2][:], xq_d[2][:])
        nc.sync.dma_start(xq_s[3][:], xq_d[3][:])
        nc.sync.dma_start(wfct_s[:], wfcd[:])
        nc.sync.dma_start(bfcb_s[:], bfcd[:])
        nc.sync.dma_start(wih0t_s[:], wih0d[:])

        # xp0 per chunk: [128, gate, t-local, b] fp32
        xp0_t = [pp.tile([128, 4, TW, BS], F32, name=f"xp0_{c}")
                 for c in range(NCHUNK)]

        # recurrence state: ew lane-major [128, lane, l, b]; lanes
        # [0:K-A, 1:C-A, 2:si-A, 3:sf-A, 4:sg-A, 5:junk, 6:so, 7:junk,
        #  8:K-B, 9:C-B, 10:si-B, 11:sf-B, 12:sg-B, 13:spare]
        hinit = pp.tile([128, 2, BS], BF16, name="hinit")
        hwin = pp.tile([128, 2, W, 2, BS], BF16, name="hwin")
        ew = pp.tile([128, 14, 2, BS], F32, name="ew")
        tcT = pp.tile([128, 2, BS], F32, name="tcT")

        nc.vector.memset(ew[:], 0.0)
        nc.vector.memset(ew[:, 0, :, :], -0.5)
        nc.vector.memset(ew[:, 8, :, :], -0.5)
        nc.vector.memset(hinit[:], 0.0)
        nc.vector.memset(wones[:], 0.0)
        # pre-warm the ACT function table (LoadActFuncSet ~1.3us) off-chain
        nc.scalar.activation(tcT[:, 0:1, :], ew[:, 1, 0:1, :], AF.Sigmoid)
        nc.scalar.activation(tcT[:, 0:1, :], ew[:, 1, 0:1, :], AF.Tanh)

        with (
            tc.tile_pool(name="xchunk", bufs=2) as x_pool,
            tc.tile_pool(name="gemm_ps", bufs=4, space="PSUM") as gemm_ps,
            tc.tile_pool(name="pair_ps", bufs=2, space="PSUM") as pair_ps,
        ):
            # ---- one chunk-0 GEMM pass: bias + k-MMs + copies ----
            def emit_pass_ops(i):
                ntok = P0TOK[i]
                toff = sum(P0TOK[:i])
                t0 = toff // BS
                tw_ = ntok // BS
                pg = []
                for g in range(4):
                    p = gemm_ps.tile([128, CTOK], F32, name="pg")
                    pg.append(p)
                    nc.tensor.matmul(p[:, 0:ntok], b0s_s[:, g, :],
                                     ones_s[:, 0:ntok],
                                     start=True, stop=False,
                                     skip_group_check=True)
                    yield 1
                for k in range(KT):
                    for g in range(4):
                        nc.tensor.matmul(
                            pg[g][:, 0:ntok], wsl(wih0f8_s, k, g),
                            xq_s[i][:, k * ntok:(k + 1) * ntok],
                            start=False, stop=(k == KT - 1),
                            skip_group_check=True)
                        yield 1
                for g in range(4):
                    dst = xp0_t[0][:, g, t0:t0 + tw_, :]
                    srcv = pg[g][:, 0:ntok].rearrange(
                        "p (t b) -> p t b", t=tw_)
                    nc.scalar.copy(dst, srcv)
                    yield 1

            # ---- GEMM op generator (pulled incrementally) ----
            def gemm_gen():
                # chunk-0 passes 1..3 (pass 0 runs in the prologue); x was
                # DMA'd up front, so only PE/copy ops are paced here
                for i in range(1, len(P0TOK)):
                    yield from emit_pass_ops(i)
                # chunks 1-3: f32r, per-k slab DMAs pace the matmuls
                for c in range(1, NCHUNK):
                    xt = x_pool.tile([128, KT, CTOK], F32R, name="xt")
                    for k in range(KT):
                        nc.sync.dma_start(xt[:, k, :], xd[c, :, k, :])
                        yield 1
                    # 256-col halves: same f32r rate (full at N>=256) but a
                    # straddling matmul blocks the chain MMs in the in-order
                    # PE queue for at most ~107ns instead of ~213ns
                    pg = []
                    for g in range(4):
                        p = gemm_ps.tile([128, CTOK], F32, name="pg")
                        pg.append(p)
                        for hh in range(2):
                            cs = slice(hh * 256, (hh + 1) * 256)
                            nc.tensor.matmul(p[:, cs], b0s_s[:, g, :],
                                             ones_s[:, cs],
                                             start=True, stop=False,
                                             skip_group_check=True)
                            yield 1
                    for k in range(KT):
                        for g in range(4):
                            for hh in range(2):
                                cs = slice(hh * 256, (hh + 1) * 256)
                                nc.tensor.matmul(
                                    pg[g][:, cs], wsl(wih0t_s, k, g),
                                    xt[:, k, cs],
                                    start=False, stop=(k == KT - 1),
                                    skip_group_check=True)
                                yield 1
                    # quarter-size copies: more total ACT busy (the ~185ns
                    # init is per-op) but each op is short enough that the
                    # scheduler can slot it into ACT idle windows without
                    # delaying the chain's sigmoid/tanh
                    for g in range(4):
                        for hh in range(32):
                            tsl = slice(hh * TW // 32, (hh + 1) * TW // 32)
                            dst = xp0_t[c][:, g, tsl, :]
                            srcv = pg[g][:, hh * 16:(hh + 1) * 16].rearrange(
                                "p (t b) -> p t b", t=TW // 32)
                            nc.scalar.copy(dst, srcv)
                            yield 1

            gen = gemm_gen()

            def pull(n):
                for _ in range(n):
                    if next(gen, None) is None:
                        break

            # ---- prologue ----
            # PE warm-up: ~4us of dummy matmuls ramp the PE out of its cold
            # p-state so pass 0 runs at full clock the moment wih0f8 lands
            warm = gemm_ps.tile([128, CTOK], F32, name="pg")
            for _ in range(16):
                nc.tensor.matmul(warm[:, 0:256], wones[0:1, 0:128],
                                 wones[:, 0:256], start=True, stop=True,
                                 skip_group_check=True)
            # pass 0 (fp8, first window) accumulates STRAIGHT INTO the
            # window-0 PSUM pair: no xp0 store, no copies, no identity
            # injection on the critical path.  Layer 0's bias lands via the
            # same K=4 onehot trick as layer 1's (ONE start=True per bank).
            P0 = pair_ps.tile([128, 2, 4, 16, BS], F32, name="pairP")
            nc.tensor.matmul(P0[:, 0, :, 0:W, :], b0s4_s[:, :],
                             onehot_s[:, :, :],
                             start=True, stop=False, skip_group_check=True)
            for k in range(KT):
                for g in range(4):
                    nc.tensor.matmul(
                        P0[:, 0, g, 0:W, :].rearrange("p s b -> p (s b)"),
                        wsl(wih0f8_s, k, g),
                        xq_s[0][:, k * W * BS:(k + 1) * W * BS],
                        start=False, stop=False, skip_group_check=True)

            P = None
            for u in range(T + LAG):
                w, s = divmod(u, W)
                if s == 0:
                    P = P0 if u == 0 else pair_ps.tile(
                        [128, 2, 4, 16, BS], F32, name="pairP")
                    if u < T and u > 0:
                        c, lw = divmod(w, TW // W)
                        nc.tensor.matmul(
                            P[:, 0, :, 0:W, :],
                            ident_s[:, :],
                            xp0_t[c][:, :, lw * W:(lw + 1) * W, :],
                            start=True, stop=False, skip_group_check=True)
                    if u + W > LAG:
                        # whole-bank bias broadcast in ONE start=True matmul
                        # (start=True clears has_written for the full bank);
                        # Wih1 @ h0 is added per-step (off the critical path).
                        nc.tensor.matmul(
                            P[:, 1, :, 0:W, :], b1s_s[:, :], onehot_s[:, :, :],
                            start=True, stop=False, skip_group_check=True)
                _build_phase2_step(nc, u, P, hwin, hinit, whh0t_s, whh1t_s,
                                   wih1t_s, ew, tcT)
                # GEMM-op interleave AFTER the step's chain matmuls: the
                # in-order PE exec queue then holds [chain MMs (sem-gated),
                # GEMM MMs], so GEMM work fills the elementwise-phase idle
                # window and never straddles the h-sem release (straddling
                # 213ns f32r matmuls cost ~4us of stragglers otherwise).
                # Start at u>=4 so the queue never stalls on a matmul whose
                # x DMA is still in flight; 8/step drains chunk-0 passes
                # 1-3 early enough for their windows while chunks 1-3 pace
                # on their slab DMAs.
                if u >= 4:
                    pull(8 if u < 32 else 6 if u < 96 else 4)
                if debug_taps and u == 31:
                    nc.sync.dma_start(dbg_xp0[:], xp0_t[0][:])
                    nc.sync.dma_start(dbg_hwin[:], hwin[:])
                    nc.sync.dma_start(dbg_c[:], cC[:])

            pull(10000)  # drain any leftovers (shouldn't be needed)

            # ---- final fc: bias folded in as a K=1 matmul (ident[0,0]
            # supplies the f32 one), y DMA'd straight from PSUM ----
            fcp = gemm_ps.tile([BS, 1], F32, name="pg")
            nc.tensor.matmul(fcp[:, :], hwin[:, (T + LAG - 1) // W % 2,
                                             (T + LAG - 1) % W, 1, :],
                             wfct_s[:, :], start=True, stop=False,
                             skip_group_check=True)
            nc.tensor.matmul(fcp[:, :], bfcb_s[:, :], ident_s[0:1, 0:1],
                             start=False, stop=True, skip_group_check=True)
            nc.vector.tensor_copy(y_sb[:, :], fcp[:, :])
            nc.sync.dma_start(yd[:], y_sb[:])

    nc.compile()
    return nc


_PROG = None


def _get_program():
    global _PROG
    if _PROG is None:
        _PROG = build_program()
    return _PROG


def prep_inputs(x, Wih0, Whh0, bih0, bhh0, Wih1, Whh1, bih1, bhh1, Wfc, bfc):
    """Host-side layout prep -> per-core in_maps."""
    bf = ml_dtypes.bfloat16
    f8 = ml_dtypes.float8_e4m3
    x = np.asarray(x, np.float32)

    # weights: [4H, K] -> [K(part), gate(ours), unit]
    def gate_T(Wmat):  # [512, K] -> [K, 4, 128] in our gate order
        A = np.asarray(Wmat, np.float32).reshape(4, 128, -1)  # tg, j, k
        A = A.transpose(2, 0, 1)[:, GORD, :]                  # k, ours, j
        A = A.copy()
        A[:, 2, :] *= 2.0  # tanh-gate folded 2x (tanh(a)=2*sig(2a)-1)
        return np.ascontiguousarray(A)

    wih0t = gate_T(Wih0).reshape(KT, 128, 4, 128).transpose(1, 0, 2, 3)
    wih0t = np.ascontiguousarray(wih0t, np.float32)           # [128,KT,4,128]
    whh0t = gate_T(Whh0).astype(bf)                           # [128,4,128]
    whh1t = gate_T(Whh1).astype(bf)
    wih1t = gate_T(Wih1).astype(bf)

    b0 = (np.asarray(bih0) + np.asarray(bhh0)).astype(np.float32)
    b1 = (np.asarray(bih1) + np.asarray(bhh1)).astype(np.float32)
    b0s = b0.reshape(4, 128)[GORD].copy()
    b0s[2] *= 2.0                                             # [4,128]
    b1g4 = b1.reshape(4, 128)[GORD].copy()
    b1g4[2] *= 2.0                                            # [4,128]
    ident = np.eye(128, dtype=np.float32)
    wfct = np.asarray(Wfc, np.float32).T.astype(bf)           # [128,1]
    bfcb = np.full((1, BS), np.asarray(bfc, np.float32)[0], np.float32)

    # packed constant blob (single startup DMA); layout must match the
    # blob_s views in build_program
    blob = np.zeros((128, BLOBW), np.float32)
    blob[:, 0:512] = whh0t.reshape(128, 512)
    blob[:, 512:1024] = whh1t.reshape(128, 512)
    blob[:, 1024:1536] = wih1t.reshape(128, 512)
    blob = blob.astype(bf)
    cbias = np.zeros((4, 1536), np.float32)
    cbias[0:4, 0:128] = b0s
    cbias[0:4, 128:384] = np.repeat(np.eye(4, dtype=np.float32),
                                    W * BS).reshape(4, 4 * W * BS)
    cbias[0, 384:896] = b0s.reshape(512)
    cbias[0, 896:1408] = 1.0
    cbias[0:4, 1408:1536] = b1g4
    cbias = cbias.astype(bf)

    wih0flat = wih0t.reshape(128, KT * 512)
    common = dict(wih0t=wih0flat,
                  cblob=blob, cbias=cbias, ident=ident, wfct=wfct,
                  bfcb=bfcb)

    offs = np.cumsum([0] + P0TOK)
    in_maps = []
    for c in range(NCORES):
        xs = x[c * BS:(c + 1) * BS]                           # [BS, T, D]
        xt = xs.transpose(2, 1, 0).reshape(D, T * BS)         # [d, tok(t,b)]
        xpre = (xt.reshape(KT, 128, NCHUNK, CTOK)
                .transpose(2, 1, 0, 3))                       # [c,128,k,tok]
        m = {"xp": np.ascontiguousarray(xpre, np.float32), **common}
        for i in range(len(P0TOK)):
            seg = np.ascontiguousarray(xpre[0][:, :, offs[i]:offs[i + 1]])
            seg = seg.astype(f8).reshape(128, KT * P0TOK[i])
            if i == 0:
                m["wx0f8"] = np.concatenate(
                    [seg, wih0flat.astype(f8)], axis=1)
            else:
                m[f"x0q{i}"] = seg
        in_maps.append(m)
    return in_maps


def run(inputs, **kw):
    nc = _get_program()
    in_maps = prep_inputs(**inputs)
    res = run_bass_kernel_spmd(nc, in_maps, core_ids=list(range(NCORES)), **kw)
    y = np.concatenate([res.results[c]["y"] for c in range(NCORES)], axis=0)
    return y.astype(np.float32), res


def kernel(**inputs):
    y, _ = run(inputs)
    return y


if __name__ == "__main__":
    import sys
    if "--sim" in sys.argv:
        import trails.perfetto as _tp
        if not hasattr(_tp.LazyPerfetto, "add_counter"):
            def _add_counter(self, proc, track, ts_, val):
                self.update_counter(proc, track, int(ts_), float(val),
                                    unit="ns")
            _tp.LazyPerfetto.add_counter = _add_counter
        for _m in ("enable_explicit_ordering", "reserve_process_order"):
            if not hasattr(_tp.LazyPerfetto, _m):
                setattr(_tp.LazyPerfetto, _m,
                        lambda self, *a, **k: None)
        from concourse.timeline_sim import TimelineSim
        nc = _get_program()
        ts = TimelineSim(nc, trace="--trace" in sys.argv)
        dur = ts.simulate()
        print(f"TimelineSim predicted duration: {dur:.0f} ns")
        if ts.perfetto is not None:
            ts.perfetto.save("/root/problem/timeline.pftrace")
            print("wrote /root/problem/timeline.pftrace")



# revision 25
# speedup vs baseline: 1.0291x; 1.0066x over previous
"""Trainium2 Bass kernel for a 2-layer LSTM binary classifier.

Model: xp0 = x @ Wih0.T + b0 ; layer0 LSTM ; xp1 = seq0 @ Wih1.T + b1 ;
layer1 LSTM ; out = h1_T @ Wfc.T + bfc.

Sharding: data-parallel over batch (64 -> 8 cores x 8 examples), all
weights replicated.  Per core:
  Phase 1 (interleaved with phase 2): big input GEMM, bias added via
    ones/onehot matmuls, output xp0 stored in SBUF as [128, gate, t, b].
    Chunk 0 (t<64) runs on fp8e4 weights (its xp0 error decays through
    >190 forget gates); later chunks use float32r (full PE rate at
    N>=256, ~tf32).  Startup is latency-tuned: HWDGE generation is ONE
    shared serialized resource (~630ns/DMA) and a DMA occupies its
    queue's SEQ until HWDGE accepts it, so all DMAs sit on the sync
    queue in priority order, constants ride in one packed blob, bulk
    tensors are flat [128,X] (the DMA cost model's descriptor unit is
    the lowest AP dim), ~4us of dummy matmuls pre-ramp the PE p-state,
    and pass 0 of the GEMM accumulates straight into the window-0 PSUM
    pair (layer-0 bias via the same K=4 onehot trick as layer 1) so the
    first recurrence step issues ~7.6us after kernel start.
  Phase 2: serial recurrence, the wall-clock driver (~1.79us/step chain
    latency x 258 macro-steps).  Gates live as [gate-dim on partitions,
    batch on free].  Per macro-step u: layer0 runs step u and layer1 runs
    step u-LAG so both layers share joint elementwise instructions.
    xp contributions are pre-accumulated into PSUM per W-step window
    (identity matmul for layer0's xp0; a single K=4 gate-onehot matmul
    broadcasts layer1's bias and must be the only start=True write to the
    bank -- start=True clears the whole bank's has_written bits).  Wih1 @
    h0(v) runs as per-step matmuls that depend on LAG-old data, so they
    execute off the critical path.  tanh-gate weights are pre-scaled 2x on
    the host so sigmoid covers the g-gate too (tanh(a) = 2*sig(2a)-1);
    the o-gate's sigmoid is a separate ACT op because it is only needed
    at the chain tail, so the on-chain sigmoid(i,f,g) is gated by just 6
    of 8 matmuls.  The 3-product cell update (tensor_mul + tensor_reduce
    originally) is ONE hand-built custom DVE op (LSTM_PAIRSUM3_ANT): a
    segmented product-scan over pages of 3 whose FSM (seed/steady/step,
    SUB_DIM_DONE reset -- the TENSOR_PAGED_MASK state-machine shape)
    resets the prefix sum at page boundaries; the out AP steers the two
    prefix elements onto junk/stale lanes and the page sum onto the
    opposite A/B state block's C lane.  Per-step chain: 6 bf16 matmuls ->
    sigmoid_ifg (ACT) -> PAIRSUM3 (DVE) -> tanh (ACT) -> h-mul (DVE,
    bf16 out) -> next step's matmuls; ~1.57us/step (was ~1.79us), every
    link at the cost model's per-op floor (ACT ops carry ~370ns of SBUF
    access latency, DVE ~120ns, PE matmuls a fixed 173ns drain).
"""

import numpy as np
import ml_dtypes

import concourse.bass as bass
import concourse.tile as tile
from concourse import bacc, mybir
from concourse.bass_utils import run_bass_kernel_spmd

# ---- custom DVE op: segmented product-sum (pages of 3) -------------------
# One Vector instruction replacing [tensor_mul(3 lanes) + tensor_reduce]:
# streams in0/in1 as [P, S, N=3] pages; per element computes p = in0*in1 and
# an inclusive per-page prefix sum (FSM: seed -> steady, SUB_DIM_DONE -> step
# resets the scan state to the current product, same state-machine shape as
# TENSOR_PAGED_MASK).  The page sum lands on the 3rd output element; the two
# prefix elements are steered to junk/stale lanes via the out AP strides.
from concourse.dve_spec import (_State, _Placement, _Stage, _assemble, PREV,
                                Spec, Src0, Src1, Zero, scan, COUNT_ONCE)
from concourse.dve_uop import (AluOp as DAlu, AluInp, Trigger, OutSel,
                               N_STAGES, DveOpSpec)
from concourse import dve_ops as _dve_ops
from concourse.dve_ops import DveOp


def _register_pairsum3():
    name = "LSTM_PAIRSUM3_ANT"
    if name in _dve_ops._SUB_OPCODE_FOR_NAME:
        return next(o for o in _dve_ops.OPS if o.name == name)
    empty = _Stage(DAlu.BYPASS, PREV)
    pipeline = [empty] * N_STAGES["v3"]
    pipeline[0] = _Stage(DAlu.MULTIPLY, Src0, Src1)
    pipeline[1] = _Stage(DAlu.ADD, AluInp.CURR_ALU_OUT, PREV)
    p = _Placement(pipeline=pipeline, node_stage={},
                   lane={Src0: 0, Src1: 1, Zero: 2},
                   out_sel=OutSel.ALU_OUT, accum_stage=None, captures=[])
    states = [
        _State(placement=p, trigger=COUNT_ONCE, repeat=1, next=(1, 0, 0),
               overrides={1: _Stage(DAlu.BYPASS, Zero, Zero)},
               write_out=False),
        _State(placement=p, consume=(True, True),
               trigger=(Trigger.SRC_TENSOR_DONE, Trigger.SUB_DIM_DONE,
                        Trigger.NONE), next=(0, 2, 0)),
        _State(placement=p, consume=(True, True), repeat=1,
               overrides={1: _Stage(DAlu.ADD, Zero, PREV)},
               trigger=(Trigger.SRC_TENSOR_DONE, Trigger.SUB_DIM_DONE,
                        Trigger.COUNT), next=(0, 2, 1)),
    ]
    uops = [_assemble(s) for s in states]
    for u in uops:
        u.validate("v3")
    opcode = max(_dve_ops._SUB_OPCODE_FOR_NAME.values()) + 1
    assert opcode < 0x20
    spec = Spec(body=scan(DAlu.ADD, Src0 * Src1))  # introspection-only
    op = DveOp(name, spec, subdim=True, uops_sha={})
    _dve_ops._COMPILE_CACHE[(name, "v3")] = DveOpSpec(
        name=name, opcode=opcode, uops=uops, rd1_en=True)
    _dve_ops.OPS.append(op)
    _dve_ops.CUSTOM_DVE_SPECS[name] = spec
    _dve_ops._SUB_OPCODE_FOR_NAME[name] = opcode
    return op


PAIRSUM3 = _register_pairsum3()

F32 = mybir.dt.float32
F32R = mybir.dt.float32r
BF16 = mybir.dt.bfloat16
F8 = mybir.dt.float8e4
AF = mybir.ActivationFunctionType

H = 128          # hidden
D = 2048         # input size
B = 64           # batch
T = 256          # seq len
NCORES = 8
BS = B // NCORES          # 8 examples per core
KT = D // 128             # 16 k-tiles of the input GEMM
NCHUNK = 4                # GEMM token chunks
CTOK = T * BS // NCHUNK   # 512 tokens per chunk
TW = CTOK // BS           # 64 timesteps per chunk
W = 8                     # recurrence window (psum burst granularity)
NW = T // W
LAG = 2                   # layer1 runs LAG steps behind layer0
GORD = [0, 1, 2, 3]       # our gate order [i,f,g,o] -> torch block index
P0TOK = [64, 64, 128, 256]  # chunk-0 GEMM pass sizes (tokens)
BLOBW = 1536              # packed-constant blob width (bf16 cols)


def _build_phase2_step(nc, u, P, hwin, hinit, whh0t_s, whh1t_s, wih1t_s,
                       ew, tcT):
    """Emit one macro-step: layer0 step u, layer1 step u-LAG."""
    w, s = divmod(u, W)
    active = []
    if u < T:
        active.append(0)
    if u >= LAG:
        active.append(1)

    def hprev(layer, step):
        if step == 0:
            return hinit[:, layer, :]
        pu = step - 1 + (LAG if layer == 1 else 0)
        return hwin[:, (pu // W) % 2, pu % W, layer, :]

    # layer1 input projection for step v=u-LAG: depends on h0(v), which was
    # produced LAG steps ago -> executes early on PE, off the critical path
    if 1 in active:
        v = u - LAG
        h0v = hwin[:, (v // W) % 2, v % W, 0, :]
        for gi in range(4):
            nc.tensor.matmul(P[:, 1, gi, s, :], wih1t_s[:, gi, :],
                             h0v, start=False, stop=False,
                             skip_group_check=True)

    # step matmuls
    for gi in (0, 1, 2, 3):
        for l in active:
            st = u if l == 0 else u - LAG
            lhs = whh0t_s if l == 0 else whh1t_s
            nc.tensor.matmul(P[:, l, gi, s, :], lhs[:, gi, :], hprev(l, st),
                             start=False, stop=True, skip_group_check=True)

    lo = active[0]
    ln = len(active)
    L = slice(lo, lo + ln)
    # g-gate weights/bias pre-scaled by 2 on host (tanh(a) = 2*sig(2a)-1)
    # and the cell state tracked as C = c/2 (exact), so the update is an
    # EQUAL-weight 3-product sum:  C' = sf*C + sg*si + si*(-0.5).
    # ew lanes (lane-major [128, lane, l, b]) double-buffer the state in A/B
    # blocks so ONE PAIRSUM3 custom op does products+sum in a single Vector
    # instruction: A = [K@0, C@1, si@2, sf@3, sg@4], B = [K@8, C@9, si@10,
    # sf@11, sg@12], so@6.  Even steps read A (in0 = lanes 2:5 overlapping
    # in1 = 0:3, the baseline trick) and the out AP (stride +2 lanes) steers
    # the two prefix elements to junk lanes 5,7 and the page sum to C-B@9;
    # odd steps read B and write (5,3,1) descending, landing C' on C-A@1.
    # Within-op write-then-read hazards: none (writes trail reads per
    # element, and junk targets are never read by the same op).
    A = (u % 2 == 0)
    gb = 2 if A else 10
    nc.scalar.activation(ew[:, gb:gb + 3, L, :],
                         P[:, L, 0:3, s, :].rearrange("p l g b -> p g l b"),
                         AF.Sigmoid)
    nc.scalar.activation(ew[:, 6, L, :], P[:, L, 3, s, :], AF.Sigmoid)
    nc.vector._custom_dve(
        PAIRSUM3,
        out=(ew[:, 5:10:2, L, :] if A
             else ew[:, 5:0:-2, L, :]).rearrange("p k l b -> p (l b) k"),
        in0=ew[:, gb:gb + 3, L, :].rearrange("p k l b -> p (l b) k"),
        in1=ew[:, gb - 2:gb + 1, L, :].rearrange("p k l b -> p (l b) k"))
    cl = 9 if A else 1
    nc.scalar.activation(tcT[:, L, :], ew[:, cl, L, :], AF.Tanh, scale=2.0)
    nc.vector.tensor_mul(hwin[:, w % 2, s, L, :], ew[:, 6, L, :],
                         tcT[:, L, :])


def build_program(debug_taps=False):
    nc = bacc.Bacc("TRN2", target_bir_lowering=False, debug=False,
                   enable_asserts=False)

    # ---- DRAM I/O ----
    xd = nc.dram_tensor("xp", [NCHUNK, 128, KT, CTOK], F32R,
                        kind="ExternalInput").ap()
    # all bulk tensors are flat [128, X] so their DMA APs have a large
    # innermost dim (the cost model's descriptor unit is the lowest AP dim)
    wih0d = nc.dram_tensor("wih0t", [128, KT * 512], F32R,
                           kind="ExternalInput").ap()
    # pass-0 x (first KT*64 cols) and the fp8 chunk-0 weights share one
    # dram tensor so the first startup DMA covers both (saves a HWDGE slot)
    wx0f8d = nc.dram_tensor("wx0f8", [128, KT * 64 + KT * 512], F8,
                            kind="ExternalInput").ap()
    xq_d = [None] + [nc.dram_tensor(f"x0q{i}", [128, KT * P0TOK[i]], F8,
                                    kind="ExternalInput").ap()
                     for i in range(1, len(P0TOK))]
    # weight blob (cols 0:512 whh0t, 512:1024 whh1t, 1024:1536 wih1t) is
    # DMA'd in two slices so whh0t lands early; every constant the first
    # recurrence window needs rides in the tiny cbias DMA (parts 0:4:
    # b0s4@0:128, onehot@128:384, b1s@1408:1536; part 0: b0s@384:896,
    # ones@896:1408) -- b1s must NOT wait on the blob, the u=0 layer-1
    # onehot matmul queues ahead of u=0's step matmuls on the PE.
    blobd = nc.dram_tensor("cblob", [128, BLOBW], BF16,
                           kind="ExternalInput").ap()
    cbiasd = nc.dram_tensor("cbias", [4, 1536], BF16,
                            kind="ExternalInput").ap()
    identd = nc.dram_tensor("ident", [128, 128], F32,
                            kind="ExternalInput").ap()
    wfcd = nc.dram_tensor("wfct", [128, 1], BF16, kind="ExternalInput").ap()
    bfcd = nc.dram_tensor("bfcb", [1, BS], F32, kind="ExternalInput").ap()
    yd = nc.dram_tensor("y", [BS, 1], F32, kind="ExternalOutput").ap()
    if debug_taps:
        dbg_xp0 = nc.dram_tensor("dbg_xp0", [128, 4, TW, BS], F32,
                                 kind="ExternalOutput").ap()
        dbg_hwin = nc.dram_tensor("dbg_hwin", [128, 2, W, 2, BS], BF16,
                                  kind="ExternalOutput").ap()
        dbg_c = nc.dram_tensor("dbg_c", [128, 2, BS], F32,
                               kind="ExternalOutput").ap()

    with tile.TileContext(nc) as tc, \
            tc.tile_pool(name="persist", bufs=1) as pp:
        # ---- persistent SBUF (bulk tensors flat, views for compute) ----
        wih0t_s = pp.tile([128, KT * 512], F32R, name="wih0t_s")
        wx0f8_s = pp.tile([128, KT * 64 + KT * 512], F8, name="wx0f8_s")
        wih0f8_s = wx0f8_s[:, KT * 64:]
        xq_s = [wx0f8_s[:, 0:KT * 64]] + [
            pp.tile([128, KT * n], F8, name=f"xq{i}_s")
            for i, n in enumerate(P0TOK) if i >= 1]
        blob_s = pp.tile([128, BLOBW], BF16, name="blob_s")
        cbias_s = pp.tile([4, 1536], BF16, name="cbias_s")
        ident_s = pp.tile([128, 128], F32, name="ident_s")
        wfct_s = pp.tile([128, 1], BF16, name="wfct_s")
        bfcb_s = pp.tile([1, BS], F32, name="bfcb_s")
        wones = pp.tile([1, 256], BF16, name="wones")
        y_sb = pp.tile([BS, 1], F32, name="y_sb")

        def wsl(wtile, k, g):  # [K=128, 128] lhsT slice of a flat w tile
            o = (k * 4 + g) * 128
            return wtile[:, o:o + 128]

        # constant views into the blob
        whh0t_s = blob_s[:, 0:512].rearrange("p (g u) -> p g u", g=4)
        whh1t_s = blob_s[:, 512:1024].rearrange("p (g u) -> p g u", g=4)
        wih1t_s = blob_s[:, 1024:1536].rearrange("p (g u) -> p g u", g=4)
        b1s_s = cbias_s[0:4, 1408:1536]
        b0s4_s = cbias_s[0:4, 0:128]
        onehot_s = cbias_s[0:4, 128:384].rearrange("p (g n) -> p g n", g=4)
        b0s_s = cbias_s[0:1, 384:896].rearrange("p (g u) -> p g u", g=4)
        ones_s = cbias_s[0:1, 896:1408]

        # Startup DMA schedule.  HWDGE generation is a single shared
        # serialized resource (~630ns/DMA) and a DMA instruction occupies
        # its queue's SEQ until HWDGE accepts it, so: recurrence-critical
        # DMAs first on sync in priority order, bulky weights on gpsimd
        # (SWDGE -- separate generator, keeps HWDGE free), and NOTHING on
        # the scalar/vector queues (their SEQs feed the recurrence).
        # Critical set for the first window: blob, wih0f8, xq0, ident.
        # single-queue DMA schedule in priority order (the DMA_ENGINES
        # transfer order is FIFO by generation-completion, so sync-queue
        # issue order IS the arrival order); chunk 0 uses the fp8 weights
        # throughout -- its xp0 error decays through >190 forget gates
        nc.sync.dma_start(cbias_s[:], cbiasd[:])
        nc.sync.dma_start(wx0f8_s[:, 0:3072], wx0f8d[:, 0:3072])
        for q in range(1, 4):
            o = KT * 64 + q * 2048
            nc.sync.dma_start(wx0f8_s[:, o:o + 2048], wx0f8d[:, o:o + 2048])
        nc.sync.dma_start(blob_s[:, 0:512], blobd[:, 0:512])
        nc.sync.dma_start(blob_s[:, 512:BLOBW], blobd[:, 512:BLOBW])
        nc.sync.dma_start(xq_s[1][:], xq_d[1][:])
        nc.sync.dma_start(ident_s[:], identd[:])
        nc.sync.dma_start(xq_s[2][:], xq_d[2][:])
        nc.sync.dma_start(xq_s[3][:], xq_d[3][:])
        nc.sync.dma_start(wfct_s[:], wfcd[:])
        nc.sync.dma_start(bfcb_s[:], bfcd[:])
        nc.sync.dma_start(wih0t_s[:], wih0d[:])

        # xp0 per chunk: [128, gate, t-local, b] fp32
        xp0_t = [pp.tile([128, 4, TW, BS], F32, name=f"xp0_{c}")
                 for c in range(NCHUNK)]

        # recurrence state: ew lane-major [128, lane, l, b]; lanes
        # [0:K-A, 1:C-A, 2:si-A, 3:sf-A, 4:sg-A, 5:junk, 6:so, 7:junk,
        #  8:K-B, 9:C-B, 10:si-B, 11:sf-B, 12:sg-B, 13:spare]
        hinit = pp.tile([128, 2, BS], BF16, name="hinit")
        hwin = pp.tile([128, 2, W, 2, BS], BF16, name="hwin")
        ew = pp.tile([128, 14, 2, BS], F32, name="ew")
        tcT = pp.tile([128, 2, BS], F32, name="tcT")

        nc.vector.memset(ew[:], 0.0)
        nc.vector.memset(ew[:, 0, :, :], -0.5)
        nc.vector.memset(ew[:, 8, :, :], -0.5)
        nc.vector.memset(hinit[:], 0.0)
        nc.vector.memset(wones[:], 0.0)
        # pre-warm the ACT function table (LoadActFuncSet ~1.3us) off-chain
        nc.scalar.activation(tcT[:, 0:1, :], ew[:, 1, 0:1, :], AF.Sigmoid)
        nc.scalar.activation(tcT[:, 0:1, :], ew[:, 1, 0:1, :], AF.Tanh)

        with (
            tc.tile_pool(name="xchunk", bufs=2) as x_pool,
            tc.tile_pool(name="gemm_ps", bufs=4, space="PSUM") as gemm_ps,
            tc.tile_pool(name="pair_ps", bufs=2, space="PSUM") as pair_ps,
        ):
            # ---- one chunk-0 GEMM pass: bias + k-MMs + copies ----
            def emit_pass_ops(i):
                ntok = P0TOK[i]
                toff = sum(P0TOK[:i])
                t0 = toff // BS
                tw_ = ntok // BS
                pg = []
                for g in range(4):
                    p = gemm_ps.tile([128, CTOK], F32, name="pg")
                    pg.append(p)
                    nc.tensor.matmul(p[:, 0:ntok], b0s_s[:, g, :],
                                     ones_s[:, 0:ntok],
                                     start=True, stop=False,
                                     skip_group_check=True)
                    yield 1
                for k in range(KT):
                    for g in range(4):
                        nc.tensor.matmul(
                            pg[g][:, 0:ntok], wsl(wih0f8_s, k, g),
                            xq_s[i][:, k * ntok:(k + 1) * ntok],
                            start=False, stop=(k == KT - 1),
                            skip_group_check=True)
                        yield 1
                for g in range(4):
                    for hh in range(tw_ // 2):
                        t0h = t0 + hh * 2
                        dst = xp0_t[0][:, g, t0h:t0h + 2, :]
                        srcv = pg[g][:, hh * 16:(hh + 1) * 16].rearrange(
                            "p (t b) -> p t b", t=2)
                        nc.scalar.copy(dst, srcv)
                        yield 1

            # ---- GEMM op generator (pulled incrementally) ----
            def gemm_gen():
                # chunk-0 passes 1..3 (pass 0 runs in the prologue); x was
                # DMA'd up front, so only PE/copy ops are paced here
                for i in range(1, len(P0TOK)):
                    yield from emit_pass_ops(i)
                # chunks 1-3: f32r, per-k slab DMAs pace the matmuls
                for c in range(1, NCHUNK):
                    xt = x_pool.tile([128, KT, CTOK], F32R, name="xt")
                    for k in range(KT):
                        nc.sync.dma_start(xt[:, k, :], xd[c, :, k, :])
                        yield 1
                    # 256-col halves: same f32r rate (full at N>=256) but a
                    # straddling matmul blocks the chain MMs in the in-order
                    # PE queue for at most ~107ns instead of ~213ns
                    pg = []
                    for g in range(4):
                        p = gemm_ps.tile([128, CTOK], F32, name="pg")
                        pg.append(p)
                        for hh in range(2):
                            cs = slice(hh * 256, (hh + 1) * 256)
                            nc.tensor.matmul(p[:, cs], b0s_s[:, g, :],
                                             ones_s[:, cs],
                                             start=True, stop=False,
                                             skip_group_check=True)
                            yield 1
                    for k in range(KT):
                        for g in range(4):
                            for hh in range(2):
                                cs = slice(hh * 256, (hh + 1) * 256)
                                nc.tensor.matmul(
                                    pg[g][:, cs], wsl(wih0t_s, k, g),
                                    xt[:, k, cs],
                                    start=False, stop=(k == KT - 1),
                                    skip_group_check=True)
                                yield 1
                    # quarter-size copies: more total ACT busy (the ~185ns
                    # init is per-op) but each op is short enough that the
                    # scheduler can slot it into ACT idle windows without
                    # delaying the chain's sigmoid/tanh
                    for g in range(4):
                        for hh in range(32):
                            tsl = slice(hh * TW // 32, (hh + 1) * TW // 32)
                            dst = xp0_t[c][:, g, tsl, :]
                            srcv = pg[g][:, hh * 16:(hh + 1) * 16].rearrange(
                                "p (t b) -> p t b", t=TW // 32)
                            nc.scalar.copy(dst, srcv)
                            yield 1

            gen = gemm_gen()

            def pull(n):
                for _ in range(n):
                    if next(gen, None) is None:
                        break

            # ---- prologue ----
            # PE warm-up: ~4us of dummy matmuls ramp the PE out of its cold
            # p-state so pass 0 runs at full clock the moment wih0f8 lands
            warm = gemm_ps.tile([128, CTOK], F32, name="pg")
            for _ in range(16):
                nc.tensor.matmul(warm[:, 0:256], wones[0:1, 0:128],
                                 wones[:, 0:256], start=True, stop=True,
                                 skip_group_check=True)
            # pass 0 (fp8, first window) accumulates STRAIGHT INTO the
            # window-0 PSUM pair: no xp0 store, no copies, no identity
            # injection on the critical path.  Layer 0's bias lands via the
            # same K=4 onehot trick as layer 1's (ONE start=True per bank).
            P0 = pair_ps.tile([128, 2, 4, 16, BS], F32, name="pairP")
            nc.tensor.matmul(P0[:, 0, :, 0:W, :], b0s4_s[:, :],
                             onehot_s[:, :, :],
                             start=True, stop=False, skip_group_check=True)
            for k in range(KT):
                for g in range(4):
                    nc.tensor.matmul(
                        P0[:, 0, g, 0:W, :].rearrange("p s b -> p (s b)"),
                        wsl(wih0f8_s, k, g),
                        xq_s[0][:, k * W * BS:(k + 1) * W * BS],
                        start=False, stop=False, skip_group_check=True)

            P = None
            for u in range(T + LAG):
                w, s = divmod(u, W)
                if s == 0:
                    P = P0 if u == 0 else pair_ps.tile(
                        [128, 2, 4, 16, BS], F32, name="pairP")
                    if u < T and u > 0:
                        c, lw = divmod(w, TW // W)
                        nc.tensor.matmul(
                            P[:, 0, :, 0:W, :],
                            ident_s[:, :],
                            xp0_t[c][:, :, lw * W:(lw + 1) * W, :],
                            start=True, stop=False, skip_group_check=True)
                    if u + W > LAG:
                        # whole-bank bias broadcast in ONE start=True matmul
                        # (start=True clears has_written for the full bank);
                        # Wih1 @ h0 is added per-step (off the critical path).
                        nc.tensor.matmul(
                            P[:, 1, :, 0:W, :], b1s_s[:, :], onehot_s[:, :, :],
                            start=True, stop=False, skip_group_check=True)
                _build_phase2_step(nc, u, P, hwin, hinit, whh0t_s, whh1t_s,
                                   wih1t_s, ew, tcT)
                # GEMM-op interleave AFTER the step's chain matmuls: the
                # in-order PE exec queue then holds [chain MMs (sem-gated),
                # GEMM MMs], so GEMM work fills the elementwise-phase idle
                # window and never straddles the h-sem release (straddling
                # 213ns f32r matmuls cost ~4us of stragglers otherwise).
                # Start at u>=4 so the queue never stalls on a matmul whose
                # x DMA is still in flight; 8/step drains chunk-0 passes
                # 1-3 early enough for their windows while chunks 1-3 pace
                # on their slab DMAs.
                if u >= 4:
                    pull(10 if u < 32 else 6 if u < 96 else 4)
                if debug_taps and u == 31:
                    nc.sync.dma_start(dbg_xp0[:], xp0_t[0][:])
                    nc.sync.dma_start(dbg_hwin[:], hwin[:])
                    nc.sync.dma_start(dbg_c[:], cC[:])

            pull(10000)  # drain any leftovers (shouldn't be needed)

            # ---- final fc: bias folded in as a K=1 matmul (ident[0,0]
            # supplies the f32 one), y DMA'd straight from PSUM ----
            fcp = gemm_ps.tile([BS, 1], F32, name="pg")
            nc.tensor.matmul(fcp[:, :], hwin[:, (T + LAG - 1) // W % 2,
                                             (T + LAG - 1) % W, 1, :],
                             wfct_s[:, :], start=True, stop=False,
                             skip_group_check=True)
            nc.tensor.matmul(fcp[:, :], bfcb_s[:, :], ident_s[0:1, 0:1],
                             start=False, stop=True, skip_group_check=True)
            nc.vector.tensor_copy(y_sb[:, :], fcp[:, :])
            nc.sync.dma_start(yd[:], y_sb[:])

    nc.compile()
    return nc


_PROG = None


def _get_program():
    global _PROG
    if _PROG is None:
        _PROG = build_program()
    return _PROG


def prep_inputs(x, Wih0, Whh0, bih0, bhh0, Wih1, Whh1, bih1, bhh1, Wfc, bfc):
    """Host-side layout prep -> per-core in_maps."""
    bf = ml_dtypes.bfloat16
    f8 = ml_dtypes.float8_e4m3
    x = np.asarray(x, np.float32)

    # weights: [4H, K] -> [K(part), gate(ours), unit]
    def gate_T(Wmat):  # [512, K] -> [K, 4, 128] in our gate order
        A = np.asarray(Wmat, np.float32).reshape(4, 128, -1)  # tg, j, k
        A = A.transpose(2, 0, 1)[:, GORD, :]                  # k, ours, j
        A = A.copy()
        A[:, 2, :] *= 2.0  # tanh-gate folded 2x (tanh(a)=2*sig(2a)-1)
        return np.ascontiguousarray(A)

    wih0t = gate_T(Wih0).reshape(KT, 128, 4, 128).transpose(1, 0, 2, 3)
    wih0t = np.ascontiguousarray(wih0t, np.float32)           # [128,KT,4,128]
    whh0t = gate_T(Whh0).astype(bf)                           # [128,4,128]
    whh1t = gate_T(Whh1).astype(bf)
    wih1t = gate_T(Wih1).astype(bf)

    b0 = (np.asarray(bih0) + np.asarray(bhh0)).astype(np.float32)
    b1 = (np.asarray(bih1) + np.asarray(bhh1)).astype(np.float32)
    b0s = b0.reshape(4, 128)[GORD].copy()
    b0s[2] *= 2.0                                             # [4,128]
    b1g4 = b1.reshape(4, 128)[GORD].copy()
    b1g4[2] *= 2.0                                            # [4,128]
    ident = np.eye(128, dtype=np.float32)
    wfct = np.asarray(Wfc, np.float32).T.astype(bf)           # [128,1]
    bfcb = np.full((1, BS), np.asarray(bfc, np.float32)[0], np.float32)

    # packed constant blob (single startup DMA); layout must match the
    # blob_s views in build_program
    blob = np.zeros((128, BLOBW), np.float32)
    blob[:, 0:512] = whh0t.reshape(128, 512)
    blob[:, 512:1024] = whh1t.reshape(128, 512)
    blob[:, 1024:1536] = wih1t.reshape(128, 512)
    blob = blob.astype(bf)
    cbias = np.zeros((4, 1536), np.float32)
    cbias[0:4, 0:128] = b0s
    cbias[0:4, 128:384] = np.repeat(np.eye(4, dtype=np.float32),
                                    W * BS).reshape(4, 4 * W * BS)
    cbias[0, 384:896] = b0s.reshape(512)
    cbias[0, 896:1408] = 1.0
    cbias[0:4, 1408:1536] = b1g4
    cbias = cbias.astype(bf)

    wih0flat = wih0t.reshape(128, KT * 512)
    common = dict(wih0t=wih0flat,
                  cblob=blob, cbias=cbias, ident=ident, wfct=wfct,
                  bfcb=bfcb)

    offs = np.cumsum([0] + P0TOK)
    in_maps = []
    for c in range(NCORES):
        xs = x[c * BS:(c + 1) * BS]                           # [BS, T, D]
        xt = xs.transpose(2, 1, 0).reshape(D, T * BS)         # [d, tok(t,b)]
        xpre = (xt.reshape(KT, 128, NCHUNK, CTOK)
                .transpose(2, 1, 0, 3))                       # [c,128,k,tok]
        m = {"xp": np.ascontiguousarray(xpre, np.float32), **common}
        for i in range(len(P0TOK)):
            seg = np.ascontiguousarray(xpre[0][:, :, offs[i]:offs[i + 1]])
            seg = seg.astype(f8).reshape(128, KT * P0TOK[i])
            if i == 0:
                m["wx0f8"] = np.concatenate(
                    [seg, wih0flat.astype(f8)], axis=1)
            else:
                m[f"x0q{i}"] = seg
        in_maps.append(m)
    return in_maps


def run(inputs, **kw):
    nc = _get_program()
    in_maps = prep_inputs(**inputs)
    res = run_bass_kernel_spmd(nc, in_maps, core_ids=list(range(NCORES)), **kw)
    y = np.concatenate([res.results[c]["y"] for c in range(NCORES)], axis=0)
    return y.astype(np.float32), res


def kernel(**inputs):
    y, _ = run(inputs)
    return y


if __name__ == "__main__":
    import sys
    if "--sim" in sys.argv:
        import trails.perfetto as _tp
        if not hasattr(_tp.LazyPerfetto, "add_counter"):
            def _add_counter(self, proc, track, ts_, val):
                self.update_counter(proc, track, int(ts_), float(val),
                                    unit="ns")
            _tp.LazyPerfetto.add_counter = _add_counter
        for _m in ("enable_explicit_ordering", "reserve_process_order"):
            if not hasattr(_tp.LazyPerfetto, _m):
                setattr(_tp.LazyPerfetto, _m,
                        lambda self, *a, **k: None)
        from concourse.timeline_sim import TimelineSim
        nc = _get_program()
        ts = TimelineSim(nc, trace="--trace" in sys.argv)
        dur = ts.simulate()
        print(f"TimelineSim predicted duration: {dur:.0f} ns")
        if ts.perfetto is not None:
            ts.perfetto.save("/root/problem/timeline.pftrace")
            print("wrote /root/problem/timeline.pftrace")

